# revision 28
# baseline (speedup 1.0000x reference)
# Deformable conv2d (offset conv -> bilinear sampling -> conv -> BN -> SiLU)
# on 8 trn2 NeuronCores, data-parallel over batch (1 image per core).
import sys

for _p in ("/opt/trn_rl_repo",):
    if _p not in sys.path:
        sys.path.insert(0, _p)

import numpy as np

import concourse.bacc as bacc
import concourse.bass as bass
import concourse.mybir as mybir
import concourse.tile as tile
from concourse.bass_utils import run_bass_kernel_spmd

F32 = mybir.dt.float32
F16 = mybir.dt.float16
I16 = mybir.dt.int16
AOT = mybir.AluOpType
AFT = mybir.ActivationFunctionType

B, CIN, H, W = 8, 128, 64, 64
COUT = 256
HW = H * W  # 4096
NT = 32     # position tiles of 128
NK = 9      # taps
TK = NT * NK
MAGIC = 12582912.0  # 1.5 * 2**23: (v + MAGIC) - MAGIC == RNE(v), |v| < 2**22
EPS = 1e-5


def build_nc(n_cores: int, dbg: bool = False, stage: int = 9,
             use_silu: bool = True):
    nc = bacc.Bacc("TRN2", target_bir_lowering=False, debug=False,
                   num_devices=n_cores, num_swdge_queues=4)

    xpad = nc.dram_tensor("xpad", [128, 66 * 66], F16, kind="ExternalInput")
    xT = nc.dram_tensor("xT", [HW, 128], F16, kind="ExternalInput")
    ow_t = nc.dram_tensor("ow_t", [NK, 128, 18], F16, kind="ExternalInput")
    ob = nc.dram_tensor("ob", [18, 1], F32, kind="ExternalInput")
    dw_t = nc.dram_tensor("dw_t", [NK, 128, COUT], F16, kind="ExternalInput")
    baseY = nc.dram_tensor("baseY", [128, TK], F32, kind="ExternalInput")
    baseX = nc.dram_tensor("baseX", [128, TK], F32, kind="ExternalInput")
    ident = nc.dram_tensor("ident", [128, 128], F16, kind="ExternalInput")
    gamma2 = nc.dram_tensor("gamma2", [128, 2], F32, kind="ExternalInput")
    beta2 = nc.dram_tensor("beta2", [128, 2], F32, kind="ExternalInput")
    yout = nc.dram_tensor("yout", [COUT, HW], F32, kind="ExternalOutput")
    cc_in = nc.dram_tensor("cc_in", [128, 4], F32)
    cc_out = nc.dram_tensor("cc_out", [128, 4], F32)

    dbg_t = {}
    if dbg:
        dbg_t["d_offT"] = nc.dram_tensor("d_offT", [128, NT * 18], F32,
                                         kind="ExternalOutput")
        dbg_t["d_w4"] = nc.dram_tensor("d_w4", [128, TK * 4], F32,
                                       kind="ExternalOutput")
        dbg_t["d_ic"] = nc.dram_tensor("d_ic", [128, NK * 64], I16,
                                       kind="ExternalOutput")
        dbg_t["d_ig"] = nc.dram_tensor("d_ig", [128, NK * 512], I16,
                                       kind="ExternalOutput")
        dbg_t["d_patT"] = nc.dram_tensor("d_patT", [128, NK * 2048], F16,
                                         kind="ExternalOutput")
        dbg_t["d_ysb"] = nc.dram_tensor("d_ysb", [128, 2 * HW], F32,
                                        kind="ExternalOutput")
        dbg_t["d_sc"] = nc.dram_tensor("d_sc", [128, 14 * TK], F32,
                                       kind="ExternalOutput")

    with tile.TileContext(nc) as tc:
        _kernel(tc, nc, n_cores, xpad=xpad, xT=xT, ow_t=ow_t, ob=ob, dw_t=dw_t,
                baseY=baseY, baseX=baseX, ident=ident, gamma2=gamma2,
                beta2=beta2, yout=yout, cc_in=cc_in, cc_out=cc_out,
                dbg_t=dbg_t, stage=stage, use_silu=use_silu)
    nc.compile()
    return nc


def _coords_half(nc, half, offT, baseY_sb, baseX_sb, sc, sc2, sc3, w4, icomp,
                 ifold, igath):
    """S3+S4 for one half (16 tiles, 144 (t,k) columns). The y-axis chain
    runs on DVE, the independent x-axis chain on GPSIMD (Pool)."""
    v = nc.vector
    gp = nc.gpsimd
    HTK = 16 * NK  # 144
    ts0, ts1 = 16 * half, 16 * (half + 1)

    offTv = offT[:].rearrange("p (t c) -> p t c", c=18)
    dy = offTv[:, ts0:ts1, 0:18:2]
    dx = offTv[:, ts0:ts1, 1:18:2]
    byv = baseY_sb[:].rearrange("p (t k) -> p t k", k=NK)[:, ts0:ts1, :]
    bxv = baseX_sb[:].rearrange("p (t k) -> p t k", k=NK)[:, ts0:ts1, :]

    def S(i):
        return sc[:, TK * i + HTK * half: TK * i + HTK * (half + 1)]

    def S2(i):
        return sc2[:, TK * i + HTK * half: TK * i + HTK * (half + 1)]

    def S3(i):
        return sc3[:, TK * i + HTK * half: TK * i + HTK * (half + 1)]

    sy, sx = S(0), S(1)
    y0, x0 = S(2), S(3)
    wy, wx = S(4), S(5)
    tA, tB = S(6), S(7)
    ay0, ay1 = S(8), S(9)
    ax0, ax1 = S(10), S(11)
    pyc, pxc = S(12), S(13)

    v.tensor_tensor(sy, dy, byv, AOT.add)
    v.tensor_tensor(sx, dx, bxv, AOT.add)

    def floorv(e, scr, dst, src):
        # dst = RNE(src - 0.5): equals floor(src) except at integer/tie
        # boundaries, where dst = floor +- 1 with frac 1.0 / ~0.0 -- the
        # slot weights then put ~all weight on the true sample point, so
        # the bilinear value error is O(ulp), not discrete.
        # NB: immediates are bf16-rounded at encode; bf16-exact consts only.
        e.tensor_scalar(scr(0), src, -0.5, None, AOT.add)
        e.tensor_scalar(dst, scr(0), MAGIC, MAGIC, AOT.add, AOT.subtract)

    floorv(v, S2, y0, sy)
    floorv(v, S3, x0, sx)
    v.tensor_tensor(wy, sy, y0, AOT.subtract)
    v.tensor_tensor(wx, sx, x0, AOT.subtract)

    def slot_weights(e, scr, w, c0, s0_out, s1_out):
        # s0 = (1-w)*[0<=c0<=62] + w*[c0==-1]
        # s1 = w*[0<=c0<=62] + (1-w)*[c0==63]
        t0, t1, t2, t3 = scr(0), scr(1), scr(2), scr(3)
        e.tensor_scalar(t0, c0, 0.0, None, AOT.is_ge)
        e.tensor_scalar(t1, c0, 62.0, None, AOT.is_le)
        e.tensor_tensor(t0, t0, t1, AOT.mult)             # m0
        e.tensor_scalar(t1, w, -1.0, 1.0, AOT.mult, AOT.add)   # 1-w
        e.tensor_scalar(t2, c0, -1.0, None, AOT.is_equal)      # sL
        e.tensor_scalar(t3, c0, 63.0, None, AOT.is_equal)      # sR
        e.tensor_tensor(t2, w, t2, AOT.mult)              # w*sL
        e.tensor_tensor(t3, t1, t3, AOT.mult)             # (1-w)*sR
        e.tensor_tensor(s0_out, t1, t0, AOT.mult)         # (1-w)*m0
        e.tensor_tensor(s0_out, s0_out, t2, AOT.add)
        e.tensor_tensor(s1_out, w, t0, AOT.mult)          # w*m0
        e.tensor_tensor(s1_out, s1_out, t3, AOT.add)

    slot_weights(v, S2, wy, y0, ay0, ay1)
    slot_weights(v, S3, wx, x0, ax0, ax1)

    v.tensor_scalar(pyc, y0, 0.0, 62.0, AOT.max, AOT.min)
    v.tensor_scalar(pxc, x0, 0.0, 62.0, AOT.max, AOT.min)
    v.tensor_scalar(tA, pyc, 64.0, None, AOT.mult)
    v.tensor_tensor(tA, tA, pxc, AOT.add)            # idxA

    # w4 [128, (t, k, corner)] slices for this half
    w4v = w4[:].rearrange("p (t k c) -> p t k c", k=NK, c=4)[:, ts0:ts1]
    for (ry, cx) in ((0, 0), (0, 1), (1, 0), (1, 1)):
        ayv = (ay0 if ry == 0 else ay1).rearrange("p (t k) -> p t k", k=NK)
        axv = (ax0 if cx == 0 else ax1).rearrange("p (t k) -> p t k", k=NK)
        v.tensor_tensor(w4v[:, :, :, 2 * ry + cx], ayv, axv, AOT.mult)

    # icomp [128, (k, t, ry)] int16, this half's t range.
    # idxB = idxA + 64 is fused into its int16 store.
    icv = icomp[:].rearrange("p (k t r) -> p k t r", t=NT, r=2)[:, :, ts0:ts1]
    tAv = tA.rearrange("p (t k) -> p t k", k=NK)
    v.tensor_copy(icv[:, :, :, 0].rearrange("p k t -> p t k"), tAv)
    v.tensor_scalar(icv[:, :, :, 1].rearrange("p k t -> p t k"), tAv,
                    64.0, None, AOT.add)

    # S4: fold this half's icomp cols -> igath half-columns + replicate.
    # icomp col c = k*64 + half*32 + blk ; igath col s = 8c + h.
    NC_ = NK * 64
    icf = icomp[:].rearrange("p (k hf b) -> p k hf b", hf=2, b=32)
    iff = ifold[0:16, :].rearrange("q (hh k hf b) -> q hh k hf b",
                                   hh=8, k=NK, hf=2)
    for h in range(8):
        eng = nc.sync if h % 2 == 0 else nc.scalar
        eng.dma_start(iff[:, h, :, half, :],
                      icf[16 * h:16 * (h + 1), :, half, :])
    igf = igath[:].rearrange("q (k hf b hh) -> q k hf b hh",
                             k=NK, hf=2, hh=8)
    # shuffle: igath[q, (k, half, b, h)] = ifold[q, (h, k, half, b)]
    v.tensor_copy(igf[0:16, :, half, :, :],
                  iff[:, :, :, half, :].rearrange("q hh k b -> q k b hh"))
    for rep in range(1, 8):
        eng = nc.sync if rep % 2 == 0 else nc.scalar
        eng.dma_start(igf[16 * rep:16 * (rep + 1), :, half, :, :],
                      igf[0:16, :, half, :, :])


def _kernel(tc, nc, n_cores, *, xpad, xT, ow_t, ob, dw_t, baseY, baseX, ident,
            gamma2, beta2, yout, cc_in, cc_out, dbg_t=None, stage=9,
            use_silu=True):
    from contextlib import ExitStack
    ctx = ExitStack()
    with ctx:
        pool = ctx.enter_context(tc.tile_pool(name="main", bufs=1))
        gtp = ctx.enter_context(tc.tile_pool(name="gt", bufs=2))
        ppp = ctx.enter_context(tc.tile_pool(name="pp", bufs=8))
        ps_off = ctx.enter_context(
            tc.tile_pool(name="ps_off", bufs=1, space="PSUM"))
        ps_offT = ctx.enter_context(
            tc.tile_pool(name="ps_offT", bufs=1, space="PSUM"))
        ps_tr = ctx.enter_context(
            tc.tile_pool(name="ps_tr", bufs=3, space="PSUM"))
        ps_y = ctx.enter_context(
            tc.tile_pool(name="ps_y", bufs=3, space="PSUM"))

        v = nc.vector
        s = nc.scalar
        g = nc.gpsimd

        # ---- constants / weights ----
        xpad_sb = pool.tile([128, 66 * 66], F16)
        nc.sync.dma_start(xpad_sb[:], xpad.ap())
        ow_sb = pool.tile([128, NK * 18], F16)    # [c, (k, o)]
        nc.sync.dma_start(
            ow_sb[:].rearrange("c (k o) -> c k o", k=NK),
            ow_t.ap().rearrange("k c o -> c k o"))
        ob_sb = pool.tile([18, 1], F32)
        nc.sync.dma_start(ob_sb[:], ob.ap())
        dw_sb = pool.tile([128, NK * COUT], F16)  # [c, (k, o)]
        nc.sync.dma_start(
            dw_sb[:].rearrange("c (k o) -> c k o", k=NK),
            dw_t.ap().rearrange("k c o -> c k o"))
        baseY_sb = pool.tile([128, TK], F32)
        nc.sync.dma_start(baseY_sb[:], baseY.ap())
        baseX_sb = pool.tile([128, TK], F32)
        nc.sync.dma_start(baseX_sb[:], baseX.ap())
        ident_sb = pool.tile([128, 128], F16)
        nc.sync.dma_start(ident_sb[:], ident.ap())
        gamma_sb = pool.tile([128, 2], F32)
        nc.sync.dma_start(gamma_sb[:], gamma2.ap())
        beta_sb = pool.tile([128, 2], F32)
        nc.sync.dma_start(beta_sb[:], beta2.ap())

        # ---- S1..S4 pipelined per half (16 tiles = 2048 positions) ----
        offC = pool.tile([18, HW], F16)
        xpv = xpad_sb[:].rearrange("p (a b) -> p a b", a=66)
        offT = pool.tile([128, NT * 18], F32)
        sc = pool.tile([128, 14 * TK], F32)
        sc2 = pool.tile([128, 4 * TK], F32)
        sc3 = pool.tile([128, 4 * TK], F32)
        w4 = pool.tile([128, TK * 4], F32)
        icomp = pool.tile([128, NK * 64], I16)
        NC_ = NK * 64
        ifold = pool.tile([16, 8 * NC_], I16)
        igath = pool.tile([128, NK * 512], I16)

        for half in range(2):
            # S1: offset conv for this half's 4 N-tiles
            for nl in range(4):
                n = 4 * half + nl
                po = ps_off.tile([18, 512], F32, tag="ps_off")
                for k in range(NK):
                    ky, kx = k // 3, k % 3
                    rhs = xpv[:, 8 * n + ky: 8 * n + ky + 8, kx: kx + 64]
                    nc.tensor.matmul(po[:], ow_sb[:, 18 * k: 18 * (k + 1)],
                                     rhs, start=(k == 0), stop=(k == NK - 1))
                s.activation(offC[:, 512 * n: 512 * (n + 1)], po[:],
                             AFT.Copy, bias=0.0)
            v.tensor_scalar(offC[:, 2048 * half: 2048 * (half + 1)],
                            offC[:, 2048 * half: 2048 * (half + 1)],
                            ob_sb[:], None, AOT.add)
            # S2: transposes for this half's 16 tiles, 4 per PSUM tile
            for tq in range(4):
                t0 = 16 * half + 4 * tq
                pt = ps_offT.tile([128, 4, 18], F16, tag="ps_offT")
                for ti in range(4):
                    nc.tensor.transpose(
                        pt[:, ti, :],
                        offC[:, 128 * (t0 + ti): 128 * (t0 + ti + 1)],
                        ident_sb[0:18, 0:18])
                v.tensor_copy(offT[:, 18 * t0: 18 * (t0 + 4)], pt[:])
            _coords_half(nc, half, offT, baseY_sb, baseX_sb, sc, sc2, sc3, w4,
                         icomp, ifold, igath)

        if stage < 2:
            yfin0 = pool.tile([128, HW], F32)
            g.memset(yfin0[:], 0.0)
            for M in range(2):
                nc.sync.dma_start(
                    bass.AP(tensor=yout, offset=M * 128 * HW,
                            ap=[[HW, 128], [1, HW]]), yfin0[:])
            return

        # gather source: xT rows with pair overlap (row q -> 256 els)
        xT_pairs = bass.AP(tensor=xT, offset=0, ap=[[128, HW - 1], [1, 256]])

        if stage < 3:
            yfin0 = pool.tile([128, HW], F32)
            g.memset(yfin0[:], 0.0)
            for M in range(2):
                nc.sync.dma_start(
                    bass.AP(tensor=yout, offset=M * 128 * HW,
                            ap=[[HW, 128], [1, HW]]), yfin0[:])
            return

        # ---- S5..S9 per half (2048 positions = 16 tiles) ----
        patT = pool.tile([128, NK * 2048], F16)
        ysb = pool.tile([128, 2 * HW], F32)
        stats = pool.tile([128, 32], F32)
        sq_scr = pool.tile([128, 512], F32)

        corners = ((0, 0), (0, 1), (1, 0), (1, 1))
        gseq = 0
        for half in range(2):
            gts = []
            for k in range(NK):
                gt = gtp.tile([128, 32, 256], F16, tag="gt")
                for q in range(4):
                    g.dma_gather(
                        gt[:, 8 * q: 8 * (q + 1), :], xT_pairs,
                        igath[:, 512 * k + 256 * half + 64 * q:
                              512 * k + 256 * half + 64 * (q + 1)],
                        1024, 1024, 256, elem_step=128,
                        queue_num=gseq % 4)
                    gseq += 1
                gts.append(gt)
            for k in range(NK if stage >= 4 else 0):
                gt = gts[k]
                for tq in range(4):       # groups of 4 tiles -> one evac
                    ptr = ps_tr.tile([128, 512], F16, tag="ptr")
                    for ti in range(4):
                        tl = 4 * tq + ti
                        t = half * 16 + tl
                        pp = ppp.tile([128, 128], F16, tag="pp")
                        wofs = (t * NK + k) * 4
                        for ci, (ry, cx) in enumerate(corners):
                            src = gt[:, 2 * tl + ry, 128 * cx: 128 * (cx + 1)]
                            wsc = w4[:, wofs + 2 * ry + cx:
                                     wofs + 2 * ry + cx + 1]
                            if ci == 0:
                                s.activation(pp[:], src, AFT.Copy, bias=0.0,
                                             scale=wsc)
                            else:
                                v.scalar_tensor_tensor(pp[:], src, wsc, pp[:],
                                                       AOT.mult, AOT.add)
                        nc.tensor.transpose(ptr[:, 128 * ti: 128 * (ti + 1)],
                                            pp[:], ident_sb[:])
                    s.activation(
                        patT[:, 2048 * k + 512 * tq:
                             2048 * k + 512 * (tq + 1)],
                        ptr[:], AFT.Copy, bias=0.0)

            for n in range(4 if stage >= 5 else 0):
                for M in range(2):
                    py_ = ps_y.tile([128, 512], F32, tag="ps_y")
                    for k in range(NK):
                        nc.tensor.matmul(
                            py_[:],
                            dw_sb[:, COUT * k + 128 * M:
                                  COUT * k + 128 * (M + 1)],
                            patT[:, 2048 * k + 512 * n:
                                 2048 * k + 512 * (n + 1)],
                            start=(k == 0), stop=(k == NK - 1))
                    ncol = half * 4 + n
                    dst = ysb[:, HW * M + 512 * ncol:
                              HW * M + 512 * (ncol + 1)]
                    s.activation(
                        dst, py_[:], AFT.Copy, bias=0.0,
                        accum_out=stats[:, 8 * M + ncol: 8 * M + ncol + 1])
                    s.activation(sq_scr[:], py_[:], AFT.Square,
                                 accum_out=stats[:, 16 + 8 * M + ncol:
                                                 16 + 8 * M + ncol + 1])

        if dbg_t:
            nc.sync.dma_start(dbg_t["d_sc"].ap(), sc[:])
            nc.sync.dma_start(dbg_t["d_offT"].ap(), offT[:])
            nc.sync.dma_start(dbg_t["d_w4"].ap(), w4[:])
            nc.sync.dma_start(dbg_t["d_ic"].ap(), icomp[:])
            nc.sync.dma_start(dbg_t["d_ig"].ap(), igath[:])
            nc.sync.dma_start(dbg_t["d_patT"].ap(), patT[:])
            nc.sync.dma_start(dbg_t["d_ysb"].ap(), ysb[:])

        if stage < 6:
            yfin0 = pool.tile([128, HW], F32)
            g.memset(yfin0[:], 0.0)
            for M in range(2):
                nc.sync.dma_start(
                    bass.AP(tensor=yout, offset=M * 128 * HW,
                            ap=[[HW, 128], [1, HW]]), yfin0[:])
            return

        # ---- S10: stats -> allreduce -> scale/shift ----
        st4 = pool.tile([128, 4], F32)
        stv = stats[:].rearrange("p (a n) -> p a n", n=8)
        for a in range(4):
            v.tensor_reduce(st4[:, a:a + 1], stv[:, a, :],
                            mybir.AxisListType.X, AOT.add)

        if n_cores > 1:
            nc.sync.dma_start(cc_in.ap(), st4[:])
            g.collective_compute(
                "AllReduce", AOT.add, replica_groups=[list(range(n_cores))],
                ins=[cc_in.ap()], outs=[cc_out.ap()])
            nc.sync.dma_start(st4[:], cc_out.ap())

        NTOT = float(n_cores * HW)
        mean2 = pool.tile([128, 2], F32)
        var2 = pool.tile([128, 2], F32)
        rstd2 = pool.tile([128, 2], F32)
        v.tensor_scalar(mean2[:], st4[:, 0:2], 1.0 / NTOT, None, AOT.mult)
        v.tensor_scalar(var2[:], st4[:, 2:4], 1.0 / NTOT, None, AOT.mult)
        v.tensor_tensor(rstd2[:], mean2[:], mean2[:], AOT.mult)
        v.tensor_tensor(var2[:], var2[:], rstd2[:], AOT.subtract)
        v.tensor_scalar(var2[:], var2[:], EPS, None, AOT.add)
        s.activation(var2[:], var2[:], AFT.Sqrt, bias=0.0)
        v.reciprocal(rstd2[:], var2[:])
        scl = pool.tile([128, 2], F32)
        sft = pool.tile([128, 2], F32)
        v.tensor_tensor(scl[:], gamma_sb[:], rstd2[:], AOT.mult)
        v.tensor_tensor(sft[:], mean2[:], scl[:], AOT.mult)
        v.tensor_tensor(sft[:], beta_sb[:], sft[:], AOT.subtract)

        # ---- S11: normalize + SiLU + output ----
        for M in range(2):
            yfin = pool.tile([128, HW], F32, tag="yfin")
            ysl = ysb[:, HW * M: HW * (M + 1)]
            if use_silu:
                s.activation(yfin[:], ysl, AFT.Silu,
                             bias=sft[:, M:M + 1], scale=scl[:, M:M + 1])
            else:  # CoreSim has no Silu; z * sigmoid(z) fallback
                zsc = gtp.tile([128, HW], F32, tag="gt")
                v.tensor_scalar(zsc[:], ysl, scl[:, M:M + 1], sft[:, M:M + 1],
                                AOT.mult, AOT.add)
                s.activation(yfin[:], zsc[:], AFT.Sigmoid, bias=0.0)
                v.tensor_tensor(yfin[:], zsc[:], yfin[:], AOT.mult)
            (nc.sync if M == 0 else nc.scalar).dma_start(
                bass.AP(tensor=yout, offset=M * 128 * HW,
                        ap=[[HW, 128], [1, HW]]),
                yfin[:])


# =========================================================
# host side
# =========================================================
_NC_CACHE = {}


def _get_nc(n_cores):
    if n_cores not in _NC_CACHE:
        _NC_CACHE[n_cores] = build_nc(n_cores)
    return _NC_CACHE[n_cores]


def make_in_maps(x, offset_w, offset_b, dconv_w, dconv_b, bn_gamma, bn_beta,
                 n_cores=8):
    x = np.asarray(x, np.float32)
    ow = np.asarray(offset_w, np.float32)
    dw = np.asarray(dconv_w, np.float32)
    ow_t = np.ascontiguousarray(
        ow.reshape(18, 128, 9).transpose(2, 1, 0)).astype(np.float16)
    dw_t = np.ascontiguousarray(
        dw.reshape(COUT, 128, 9).transpose(2, 1, 0)).astype(np.float16)
    ob = np.asarray(offset_b, np.float32).reshape(18, 1).copy()
    p = np.arange(128)
    t = np.arange(NT)
    k = np.arange(NK)
    ky, kx = k // 3, k % 3
    baseY = ((t[None, :, None] * 2 + (p[:, None, None] // 64)) - 1
             + ky[None, None, :]).reshape(128, TK).astype(np.float32)
    baseX = (((p[:, None, None] % 64)) - 1
             + kx[None, None, :] + 0 * t[None, :, None]).reshape(
                 128, TK).astype(np.float32)
    baseY = np.ascontiguousarray(baseY)
    baseX = np.ascontiguousarray(baseX)
    ident = np.eye(128, dtype=np.float16)
    gamma2 = np.ascontiguousarray(
        np.asarray(bn_gamma, np.float32).reshape(2, 128).T)
    beta2 = np.ascontiguousarray(
        np.asarray(bn_beta, np.float32).reshape(2, 128).T)

    in_maps = []
    for c in range(n_cores):
        xb = x[c]
        xp = np.zeros((128, 66, 66), np.float16)
        xp[:, 1:65, 1:65] = xb.astype(np.float16)
        xT = np.ascontiguousarray(xb.reshape(128, HW).T.astype(np.float16))
        in_maps.append({
            "xpad": np.ascontiguousarray(xp.reshape(128, 66 * 66)),
            "xT": xT,
            "ow_t": ow_t, "ob": ob, "dw_t": dw_t,
            "baseY": baseY, "baseX": baseX, "ident": ident,
            "gamma2": gamma2, "beta2": beta2,
        })
    return in_maps


def kernel(x, offset_w, offset_b, dconv_w, dconv_b, bn_gamma, bn_beta,
           trace=False):
    n_cores = 8
    nc = _get_nc(n_cores)
    in_maps = make_in_maps(x, offset_w, offset_b, dconv_w, dconv_b,
                           bn_gamma, bn_beta, n_cores)
    res = run_bass_kernel_spmd(nc, in_maps, list(range(n_cores)), trace=trace)
    out = np.stack([res.results[c]["yout"].reshape(COUT, H, W)
                    for c in range(n_cores)])
    kernel.last_result = res
    return out.astype(np.float32)



# revision 31
# speedup vs baseline: 1.3355x; 1.3355x over previous
# Deformable conv2d (offset conv -> bilinear sampling -> conv -> BN -> SiLU)
# on 8 trn2 NeuronCores, data-parallel over batch (1 image per core).
#
# v2: single 512B gather descriptor per (tap, position) fetching all 4
# bilinear corners from an fp8e3 "pair" image layout xtp[p] =
# [ch(p), ch(p+64)]; a descriptor covers rows p, p+1 = corners
# (y0,x0),(y1,x0),(y0,x1),(y1,x1). Blend = scalar_tensor_tensor chains
# spread across DVE / Act / Pool engines.
import sys

for _p in ("/opt/trn_rl_repo",):
    if _p not in sys.path:
        sys.path.insert(0, _p)

import numpy as np

import concourse.bacc as bacc
import concourse.bass as bass
import concourse.mybir as mybir
import concourse.tile as tile
from concourse.bass_utils import run_bass_kernel_spmd

F32 = mybir.dt.float32
F16 = mybir.dt.float16
F8E3 = mybir.dt.float8e3
I16 = mybir.dt.int16
AOT = mybir.AluOpType
AFT = mybir.ActivationFunctionType

B, CIN, H, W = 8, 128, 64, 64
COUT = 256
HW = H * W  # 4096
NT = 32     # position tiles of 128
NK = 9      # taps
TK = NT * NK
MAGIC = 12582912.0  # 1.5 * 2**23: (v + MAGIC) - MAGIC == RNE(v), |v| < 2**22
EPS = 1e-5

GATHER_FP8 = True          # gather source dtype (fp8e3 vs f16)
POOL_TILES = 0             # of 16 tiles per (k, half): last N on Pool engine
GDT = F8E3 if GATHER_FP8 else F16


def build_nc(n_cores: int, dbg: bool = False, use_silu: bool = True):
    nc = bacc.Bacc("TRN2", target_bir_lowering=False, debug=False,
                   num_devices=n_cores, num_swdge_queues=4)

    xpad = nc.dram_tensor("xpad", [128, 66 * 66], F16, kind="ExternalInput")
    # pair image: row p = [ch(p), ch(p+64)]; desc = rows p,p+1 (4 corners)
    xtp = nc.dram_tensor("xtp", [HW, 256], GDT, kind="ExternalInput")
    ow_t = nc.dram_tensor("ow_t", [NK, 128, 18], F16, kind="ExternalInput")
    ob = nc.dram_tensor("ob", [18, 1], F32, kind="ExternalInput")
    dw_t = nc.dram_tensor("dw_t", [NK, 128, COUT], F16, kind="ExternalInput")
    baseY = nc.dram_tensor("baseY", [128, TK], F32, kind="ExternalInput")
    baseX = nc.dram_tensor("baseX", [128, TK], F32, kind="ExternalInput")
    ident = nc.dram_tensor("ident", [128, 128], F16, kind="ExternalInput")
    gamma2 = nc.dram_tensor("gamma2", [128, 2], F32, kind="ExternalInput")
    beta2 = nc.dram_tensor("beta2", [128, 2], F32, kind="ExternalInput")
    yout = nc.dram_tensor("yout", [COUT, HW], F16, kind="ExternalOutput")
    cc_in = nc.dram_tensor("cc_in", [128, 4], F32)
    cc_out = nc.dram_tensor("cc_out", [128, 4], F32)

    dbg_t = {}
    if dbg:
        dbg_t["d_offT"] = nc.dram_tensor("d_offT", [128, NT * 18], F32,
                                         kind="ExternalOutput")
        dbg_t["d_w4"] = nc.dram_tensor("d_w4", [128, TK * 4], F32,
                                       kind="ExternalOutput")
        dbg_t["d_ic"] = nc.dram_tensor("d_ic", [128, TK], I16,
                                       kind="ExternalOutput")
        dbg_t["d_ig"] = nc.dram_tensor("d_ig", [128, NK * 256], I16,
                                       kind="ExternalOutput")
        dbg_t["d_patT"] = nc.dram_tensor("d_patT", [128, NK * 2048], F16,
                                         kind="ExternalOutput")
        dbg_t["d_ysb"] = nc.dram_tensor("d_ysb", [128, 2 * HW], F16,
                                        kind="ExternalOutput")

    with tile.TileContext(nc) as tc:
        _kernel(tc, nc, n_cores, xpad=xpad, xtp=xtp, ow_t=ow_t, ob=ob,
                dw_t=dw_t, baseY=baseY, baseX=baseX, ident=ident,
                gamma2=gamma2, beta2=beta2, yout=yout, cc_in=cc_in,
                cc_out=cc_out, dbg_t=dbg_t, use_silu=use_silu)
    nc.compile()
    return nc


def _coords_half(nc, half, offT, baseY_sb, baseX_sb, sc, sc2, sc3, w4, icomp):
    """S3 for one half (16 tiles, 144 (t,k) columns): sample coords ->
    4-corner weights w4 and gather index icomp = pyc*64 + pxc.
    y-axis chain on DVE, x-axis chain on Pool."""
    v = nc.vector
    g = nc.gpsimd
    HTK = 16 * NK  # 144
    ts0, ts1 = 16 * half, 16 * (half + 1)

    offTv = offT[:].rearrange("p (t c) -> p t c", c=18)
    dy = offTv[:, ts0:ts1, 0:18:2]
    dx = offTv[:, ts0:ts1, 1:18:2]
    byv = baseY_sb[:].rearrange("p (t k) -> p t k", k=NK)[:, ts0:ts1, :]
    bxv = baseX_sb[:].rearrange("p (t k) -> p t k", k=NK)[:, ts0:ts1, :]

    def S(i):
        return sc[:, TK * i + HTK * half: TK * i + HTK * (half + 1)]

    def S2(i):
        return sc2[:, TK * i + HTK * half: TK * i + HTK * (half + 1)]

    def S3(i):
        return sc3[:, TK * i + HTK * half: TK * i + HTK * (half + 1)]

    sy, sx = S(0), S(1)
    y0, x0 = S(2), S(3)
    wy, wx = S(4), S(5)
    ay0, ay1 = S(6), S(7)
    ax0, ax1 = S(8), S(9)
    pyc, pxc = S(10), S(11)

    v.tensor_tensor(sy, dy, byv, AOT.add)
    g.tensor_tensor(sx, dx, bxv, AOT.add)

    def floorv(e, scr, dst, src):
        # dst = RNE(src - 0.5): equals floor(src) except at integer/tie
        # boundaries, where dst = floor +- 1 with frac 1.0 / ~0.0 -- the
        # slot weights then put ~all weight on the true sample point, so
        # the bilinear value error is O(ulp), not discrete.
        # NB: immediates are bf16-rounded at encode; bf16-exact consts only.
        e.tensor_scalar(scr(0), src, -0.5, None, AOT.add)
        e.tensor_scalar(dst, scr(0), MAGIC, MAGIC, AOT.add, AOT.subtract)

    floorv(v, S2, y0, sy)
    floorv(g, S3, x0, sx)
    v.tensor_tensor(wy, sy, y0, AOT.subtract)
    g.tensor_tensor(wx, sx, x0, AOT.subtract)

    def slot_weights(e, scr, w, c0, s0_out, s1_out):
        # s0 = (1-w)*[0<=c0<=62] + w*[c0==-1]
        # s1 = w*[0<=c0<=62] + (1-w)*[c0==63]
        t0, t1, t2, t3 = scr(0), scr(1), scr(2), scr(3)
        e.tensor_scalar(t0, c0, 0.0, None, AOT.is_ge)
        e.tensor_scalar(t1, c0, 62.0, None, AOT.is_le)
        e.tensor_tensor(t0, t0, t1, AOT.mult)             # m0
        e.tensor_scalar(t1, w, -1.0, 1.0, AOT.mult, AOT.add)   # 1-w
        e.tensor_scalar(t2, c0, -1.0, None, AOT.is_equal)      # sL
        e.tensor_scalar(t3, c0, 63.0, None, AOT.is_equal)      # sR
        e.tensor_tensor(t2, w, t2, AOT.mult)              # w*sL
        e.tensor_tensor(t3, t1, t3, AOT.mult)             # (1-w)*sR
        e.tensor_tensor(s0_out, t1, t0, AOT.mult)         # (1-w)*m0
        e.tensor_tensor(s0_out, s0_out, t2, AOT.add)
        e.tensor_tensor(s1_out, w, t0, AOT.mult)          # w*m0
        e.tensor_tensor(s1_out, s1_out, t3, AOT.add)

    slot_weights(v, S2, wy, y0, ay0, ay1)
    slot_weights(g, S3, wx, x0, ax0, ax1)

    v.tensor_scalar(pyc, y0, 0.0, 62.0, AOT.max, AOT.min)
    g.tensor_scalar(pxc, x0, 0.0, 62.0, AOT.max, AOT.min)
    # idx = pyc*64 + pxc -> icomp (int16), cols (k, t) for this half
    tA = S2(0)
    v.tensor_scalar(tA, pyc, 64.0, None, AOT.mult)
    v.tensor_tensor(tA, tA, pxc, AOT.add)
    icv = icomp[:].rearrange("p (k t) -> p k t", t=NT)[:, :, ts0:ts1]
    tAv = tA.rearrange("p (t k) -> p t k", k=NK)
    v.tensor_copy(icv.rearrange("p k t -> p t k"), tAv)

    # w4 [128, (t, k, corner)], corner order (x-slot, y-slot):
    # 0:(x0,y0) 1:(x0,y1) 2:(x1,y0) 3:(x1,y1)
    w4v = w4[:].rearrange("p (t k c) -> p t k c", k=NK, c=4)[:, ts0:ts1]
    for ci, (ax_, ay_) in enumerate(((ax0, ay0), (ax0, ay1),
                                     (ax1, ay0), (ax1, ay1))):
        axv = ax_.rearrange("p (t k) -> p t k", k=NK)
        ayv = ay_.rearrange("p (t k) -> p t k", k=NK)
        eng = nc.vector if ci % 2 == 0 else nc.gpsimd
        eng.tensor_tensor(w4v[:, :, :, ci], axv, ayv, AOT.mult)


def _igath_half(nc, half, icomp, ifold, igath):
    """Fold icomp columns for this half into the wrapped+replicated gather
    index layout igath[q, (half, k, b, h)] (q in 16, b in 16, h in 8):
    gather j = b*128 + (h*16+q) reads igath[q, (half,k, b*8+h)]."""
    icv = icomp[:].rearrange("p (k t) -> p k t", t=NT)
    iff = ifold[0:16, :].rearrange("q (hf h k b) -> q hf h k b",
                                  hf=2, h=8, k=NK)
    for h in range(8):
        nc.sync.dma_start(iff[:, half, h, :, :],
                          icv[16 * h: 16 * (h + 1), :, 16 * half:
                              16 * (half + 1)])
    igf = igath[:].rearrange("q (hf k b h) -> q hf k b h", hf=2, k=NK, h=8)
    nc.vector.tensor_copy(igf[0:16, half],
                          iff[:, half].rearrange("q h k b -> q k b h"))
    for rep in range(1, 8):
        nc.sync.dma_start(igf[16 * rep: 16 * (rep + 1), half],
                          igf[0:16, half])


def _kernel(tc, nc, n_cores, *, xpad, xtp, ow_t, ob, dw_t, baseY, baseX,
            ident, gamma2, beta2, yout, cc_in, cc_out, dbg_t=None,
            use_silu=True):
    from contextlib import ExitStack
    ctx = ExitStack()
    with ctx:
        pool = ctx.enter_context(tc.tile_pool(name="main", bufs=1))
        gtp = ctx.enter_context(tc.tile_pool(name="gt", bufs=3))
        ppp = ctx.enter_context(tc.tile_pool(name="pp", bufs=8))
        patp = ctx.enter_context(tc.tile_pool(name="patp", bufs=2))
        ps_off = ctx.enter_context(
            tc.tile_pool(name="ps_off", bufs=1, space="PSUM"))
        ps_offT = ctx.enter_context(
            tc.tile_pool(name="ps_offT", bufs=1, space="PSUM"))
        ps_tr = ctx.enter_context(
            tc.tile_pool(name="ps_tr", bufs=4, space="PSUM"))
        ps_y = ctx.enter_context(
            tc.tile_pool(name="ps_y", bufs=2, space="PSUM"))

        v = nc.vector
        s = nc.scalar
        g = nc.gpsimd

        # ---- constants / weights ----
        # xpad lives in the gather-tile ring: it is dead after the offset
        # conv, so its slot recycles for gt tiles.
        xpad_sb = gtp.tile([128, 66 * 66], F16, tag="gt")
        nc.sync.dma_start(xpad_sb[:], xpad.ap())
        ow_sb = pool.tile([128, NK * 18], F16)    # [c, (k, o)]
        nc.sync.dma_start(
            ow_sb[:].rearrange("c (k o) -> c k o", k=NK),
            ow_t.ap().rearrange("k c o -> c k o"))
        ob_sb = pool.tile([18, 1], F32)
        nc.sync.dma_start(ob_sb[:], ob.ap())
        dw_sb = pool.tile([128, NK * COUT], F16)  # [c, (k, o)]
        nc.sync.dma_start(
            dw_sb[:].rearrange("c (k o) -> c k o", k=NK),
            dw_t.ap().rearrange("k c o -> c k o"))
        baseY_sb = pool.tile([128, TK], F32)
        nc.sync.dma_start(baseY_sb[:], baseY.ap())
        baseX_sb = pool.tile([128, TK], F32)
        nc.sync.dma_start(baseX_sb[:], baseX.ap())
        ident_sb = pool.tile([128, 128], F16)
        nc.sync.dma_start(ident_sb[:], ident.ap())
        gamma_sb = pool.tile([128, 2], F32)
        nc.sync.dma_start(gamma_sb[:], gamma2.ap())
        beta_sb = pool.tile([128, 2], F32)
        nc.sync.dma_start(beta_sb[:], beta2.ap())

        # ---- persistent tiles ----
        offC = pool.tile([18, HW], F16)
        xpv = xpad_sb[:].rearrange("p (a b) -> p a b", a=66)
        offT = pool.tile([128, NT * 18], F32)
        sc = pool.tile([128, 12 * TK], F32)
        sc2 = pool.tile([128, 4 * TK], F32)
        sc3 = pool.tile([128, 4 * TK], F32)
        w4 = pool.tile([128, TK * 4], F32)
        icomp = pool.tile([128, TK], I16)   # idx per (k, t)
        ifold = pool.tile([16, 2 * 8 * NK * 16], I16)
        igath = pool.tile([128, 2 * NK * 128], I16)
        ysb = pool.tile([128, 2 * HW], F16)
        stats = pool.tile([128, 32], F32)
        sq_scr = pool.tile([128, 512], F16)

        # gather source: xtp rows (p, p+1) -> 512 elements = 4 corners
        xtp_pairs = bass.AP(tensor=xtp, offset=0, ap=[[256, HW - 1], [1, 512]])

        # ---- phase A: S1/S2/S3/S4 for BOTH halves up front, so the PE
        # stream never blocks half-1 coords behind half-0 matmuls ----
        for half in range(2):
            # S1: offset conv for this half's 4 N-tiles (bias via evac)
            for nl in range(4):
                n = 4 * half + nl
                po = ps_off.tile([18, 512], F32, tag="ps_off")
                for k in range(NK):
                    ky, kx = k // 3, k % 3
                    rhs = xpv[:, 8 * n + ky: 8 * n + ky + 8, kx: kx + 64]
                    nc.tensor.matmul(po[:], ow_sb[:, 18 * k: 18 * (k + 1)],
                                     rhs, start=(k == 0), stop=(k == NK - 1))
                s.activation(offC[:, 512 * n: 512 * (n + 1)], po[:],
                             AFT.Identity, bias=ob_sb[:, 0:1])
            # S2: transposes for this half's 16 tiles, 4 per PSUM tile
            for tq in range(4):
                t0 = 16 * half + 4 * tq
                pt = ps_offT.tile([128, 4, 18], F16, tag="ps_offT")
                for ti in range(4):
                    nc.tensor.transpose(
                        pt[:, ti, :],
                        offC[:, 128 * (t0 + ti): 128 * (t0 + ti + 1)],
                        ident_sb[0:18, 0:18])
                v.tensor_copy(offT[:, 18 * t0: 18 * (t0 + 4)], pt[:])
            # S3 coords + S4 index build
            _coords_half(nc, half, offT, baseY_sb, baseX_sb, sc, sc2, sc3,
                         w4, icomp)
            _igath_half(nc, half, icomp, ifold, igath)

        # ---- phase B: per (half, tap): gather -> diag-build -> 4
        # accumulating diag matmuls (blend + transpose in one PE pass,
        # reading fp8 directly) -> evac. Half-0 main matmuls interleave
        # into half-1's tap loop to keep the PE stream dense. ----
        w4v = w4[:].rearrange("p (t k c) -> p t k c", k=NK, c=4)
        patTs = []
        gseq = 0

        def mm_group(half, patT, gi):
            n, M = gi // 2, gi % 2
            py_ = ps_y.tile([128, 512], F32, tag="ps_y")
            for k in range(NK):
                nc.tensor.matmul(
                    py_[:],
                    dw_sb[:, COUT * k + 128 * M: COUT * k + 128 * (M + 1)],
                    patT[:, 2048 * k + 512 * n: 2048 * k + 512 * (n + 1)],
                    start=(k == 0), stop=(k == NK - 1))
            ncol = half * 4 + n
            dst = ysb[:, HW * M + 512 * ncol: HW * M + 512 * (ncol + 1)]
            s.activation(dst, py_[:], AFT.Copy, bias=0.0,
                         accum_out=stats[:, 8 * M + ncol: 8 * M + ncol + 1])
            s.activation(sq_scr[:], py_[:], AFT.Square,
                         accum_out=stats[:, 16 + 8 * M + ncol:
                                         16 + 8 * M + ncol + 1])

        def issue_gather(half, k, qn):
            # 2 x 1024-idx calls: a single 2048-idx SWDGE gather crashes HW
            gt = gtp.tile([128, 16, 512], GDT, tag="gt")
            base = (half * NK + k) * 128
            for hh in range(2):
                g.dma_gather(gt[:, 8 * hh: 8 * (hh + 1), :], xtp_pairs,
                             igath[:, base + 64 * hh: base + 64 * (hh + 1)],
                             1024, 1024, 512, elem_step=256,
                             queue_num=(qn * 2 + hh) % 4)
            return gt

        seq = [(h, k) for h in range(2) for k in range(NK)]
        gts = {0: issue_gather(*seq[0], 0)}
        for i, (half, k) in enumerate(seq):
            if k == 0:
                patT = patp.tile([128, NK * 2048], F16, tag="patT")
                patTs.append(patT)
            if i + 1 < len(seq):
                gts[i + 1] = issue_gather(*seq[i + 1], i + 1)
            gt = gts.pop(i)
            if True:
                for tq in range(4):       # groups of 4 tiles per PSUM bank
                    ptr = ps_tr.tile([128, 512], F32, tag="ptr")
                    for ti in range(4):
                        tl = 4 * tq + ti
                        t = half * 16 + tl
                        wsl = w4v[:, t, k, :]
                        deng = s if tl == 6 else v
                        D = ppp.tile([128, 4, 128], F16, tag="pp")
                        for ci in range(4):
                            if deng is s:
                                s.activation(D[:, ci, :], ident_sb[:],
                                             AFT.Copy, bias=0.0,
                                             scale=wsl[:, ci: ci + 1])
                            else:
                                deng.tensor_scalar(D[:, ci, :], ident_sb[:],
                                                   wsl[:, ci: ci + 1], None,
                                                   AOT.mult)
                        for ci in range(4):
                            nc.tensor.matmul(
                                ptr[:, 128 * ti: 128 * (ti + 1)],
                                gt[:, tl, 128 * ci: 128 * (ci + 1)],
                                D[:, ci, :],
                                start=(ci == 0), stop=(ci == 3))
                    s.activation(
                        patT[:, 2048 * k + 512 * tq:
                             2048 * k + 512 * tq + 512],
                        ptr[:], AFT.Copy, bias=0.0)
                if half == 1 and k >= 1:
                    mm_group(0, patTs[0], k - 1)
        for gi in range(8):
            mm_group(1, patTs[1], gi)

            if dbg_t and half == 1:
                nc.sync.dma_start(dbg_t["d_patT"].ap(), patT[:])

        if dbg_t:
            nc.sync.dma_start(dbg_t["d_offT"].ap(), offT[:])
            nc.sync.dma_start(dbg_t["d_w4"].ap(), w4[:])
            nc.sync.dma_start(dbg_t["d_ic"].ap(), icomp[:])
            nc.sync.dma_start(dbg_t["d_ig"].ap(), igath[0:128, :])
            nc.sync.dma_start(dbg_t["d_ysb"].ap(), ysb[:])

        # ---- S10: stats -> allreduce -> scale/shift ----
        st4 = pool.tile([128, 4], F32)
        stv = stats[:].rearrange("p (a n) -> p a n", n=8)
        for a in range(4):
            v.tensor_reduce(st4[:, a:a + 1], stv[:, a, :],
                            mybir.AxisListType.X, AOT.add)

        if n_cores > 1:
            nc.sync.dma_start(cc_in.ap(), st4[:])
            g.collective_compute(
                "AllReduce", AOT.add, replica_groups=[list(range(n_cores))],
                ins=[cc_in.ap()], outs=[cc_out.ap()])
            nc.sync.dma_start(st4[:], cc_out.ap())

        NTOT = float(n_cores * HW)
        mean2 = pool.tile([128, 2], F32)
        var2 = pool.tile([128, 2], F32)
        rstd2 = pool.tile([128, 2], F32)
        nsc = pool.tile([128, 2], F32)
        v.tensor_scalar(mean2[:], st4[:, 0:2], 1.0 / NTOT, None, AOT.mult)
        v.tensor_scalar(var2[:], st4[:, 2:4], 1.0 / NTOT, None, AOT.mult)
        v.tensor_tensor(rstd2[:], mean2[:], mean2[:], AOT.mult)
        v.tensor_tensor(var2[:], var2[:], rstd2[:], AOT.subtract)
        v.tensor_scalar(var2[:], var2[:], EPS, None, AOT.add)
        # rstd = 1/sqrt(var) via Newton (avoids act-table reload for Sqrt):
        # x <- x*(1.5 - 0.5*v*x^2), seed 0.25 converges for var in (0, 48)
        v.memset(rstd2[:], 0.25)
        for _ in range(6):
            v.tensor_tensor(nsc[:], rstd2[:], rstd2[:], AOT.mult)
            v.tensor_tensor(nsc[:], nsc[:], var2[:], AOT.mult)
            v.tensor_scalar(nsc[:], nsc[:], -0.5, 1.5, AOT.mult, AOT.add)
            v.tensor_tensor(rstd2[:], rstd2[:], nsc[:], AOT.mult)
        scl = pool.tile([128, 2], F32)
        sft = pool.tile([128, 2], F32)
        v.tensor_tensor(scl[:], gamma_sb[:], rstd2[:], AOT.mult)
        v.tensor_tensor(sft[:], mean2[:], scl[:], AOT.mult)
        v.tensor_tensor(sft[:], beta_sb[:], sft[:], AOT.subtract)

        # ---- S11: normalize + SiLU + output ----
        for M in range(2):
            yfin = pool.tile([128, HW], F16, tag="yfin")
            ysl = ysb[:, HW * M: HW * (M + 1)]
            if use_silu:
                s.activation(yfin[:], ysl, AFT.Silu,
                             bias=sft[:, M:M + 1], scale=scl[:, M:M + 1])
            else:  # CoreSim has no Silu; z * sigmoid(z) fallback
                zsc = gtp.tile([128, HW], F32, tag="gt")
                v.tensor_scalar(zsc[:], ysl, scl[:, M:M + 1], sft[:, M:M + 1],
                                AOT.mult, AOT.add)
                s.activation(yfin[:], zsc[:], AFT.Sigmoid, bias=0.0)
                v.tensor_tensor(yfin[:], zsc[:], yfin[:], AOT.mult)
            (nc.sync if M == 0 else nc.scalar).dma_start(
                bass.AP(tensor=yout, offset=M * 128 * HW,
                        ap=[[HW, 128], [1, HW]]),
                yfin[:])


# =========================================================
# host side
# =========================================================
_NC_CACHE = {}


def _get_nc(n_cores):
    if n_cores not in _NC_CACHE:
        _NC_CACHE[n_cores] = build_nc(n_cores)
    return _NC_CACHE[n_cores]


def make_in_maps(x, offset_w, offset_b, dconv_w, dconv_b, bn_gamma, bn_beta,
                 n_cores=8):
    x = np.asarray(x, np.float32)
    ow = np.asarray(offset_w, np.float32)
    dw = np.asarray(dconv_w, np.float32)
    ow_t = np.ascontiguousarray(
        ow.reshape(18, 128, 9).transpose(2, 1, 0)).astype(np.float16)
    dw_t = np.ascontiguousarray(
        dw.reshape(COUT, 128, 9).transpose(2, 1, 0)).astype(np.float16)
    ob = np.asarray(offset_b, np.float32).reshape(18, 1).copy()
    p = np.arange(128)
    t = np.arange(NT)
    k = np.arange(NK)
    ky, kx = k // 3, k % 3
    baseY = ((t[None, :, None] * 2 + (p[:, None, None] // 64)) - 1
             + ky[None, None, :]).reshape(128, TK).astype(np.float32)
    baseX = (((p[:, None, None] % 64)) - 1
             + kx[None, None, :] + 0 * t[None, :, None]).reshape(
                 128, TK).astype(np.float32)
    baseY = np.ascontiguousarray(baseY)
    baseX = np.ascontiguousarray(baseX)
    ident = np.eye(128, dtype=np.float16)
    gamma2 = np.ascontiguousarray(
        np.asarray(bn_gamma, np.float32).reshape(2, 128).T)
    beta2 = np.ascontiguousarray(
        np.asarray(bn_beta, np.float32).reshape(2, 128).T)
    gnp = mybir.dt.np(GDT)

    in_maps = []
    for c in range(n_cores):
        xb = x[c]
        xp = np.zeros((128, 66, 66), np.float16)
        xp[:, 1:65, 1:65] = xb.astype(np.float16)
        xT = np.ascontiguousarray(xb.reshape(128, HW).T)  # [pos, ch] f32
        xtp = np.zeros((HW, 256), np.float32)
        xtp[:, 0:128] = xT
        xtp[:-64, 128:256] = xT[64:]
        in_maps.append({
            "xpad": np.ascontiguousarray(xp.reshape(128, 66 * 66)),
            "xtp": xtp.astype(gnp),
            "ow_t": ow_t, "ob": ob, "dw_t": dw_t,
            "baseY": baseY, "baseX": baseX, "ident": ident,
            "gamma2": gamma2, "beta2": beta2,
        })
    return in_maps


def kernel(x, offset_w, offset_b, dconv_w, dconv_b, bn_gamma, bn_beta,
           trace=False):
    n_cores = 8
    nc = _get_nc(n_cores)
    in_maps = make_in_maps(x, offset_w, offset_b, dconv_w, dconv_b,
                           bn_gamma, bn_beta, n_cores)
    res = run_bass_kernel_spmd(nc, in_maps, list(range(n_cores)), trace=trace)
    out = np.stack([res.results[c]["yout"].reshape(COUT, H, W)
                    for c in range(n_cores)])
    kernel.last_result = res
    return out.astype(np.float32)


# revision 58
# speedup vs baseline: 1.5627x; 1.1701x over previous
# Deformable conv2d (offset conv -> bilinear sampling -> conv -> BN -> SiLU)
# on 8 trn2 NeuronCores, data-parallel over batch (1 image per core).
#
# v2: single 512B gather descriptor per (tap, position) fetching all 4
# bilinear corners from an fp8e3 "pair" image layout xtp[p] =
# [ch(p), ch(p+64)]; a descriptor covers rows p, p+1 = corners
# (y0,x0),(y1,x0),(y0,x1),(y1,x1). Blend = scalar_tensor_tensor chains
# spread across DVE / Act / Pool engines.
import sys

for _p in ("/opt/trn_rl_repo",):
    if _p not in sys.path:
        sys.path.insert(0, _p)

import numpy as np

import concourse.bacc as bacc
import concourse.bass as bass
import concourse.mybir as mybir
import concourse.tile as tile
from concourse.bass_utils import run_bass_kernel_spmd

F32 = mybir.dt.float32
F16 = mybir.dt.float16
F8E3 = mybir.dt.float8e3
I16 = mybir.dt.int16
AOT = mybir.AluOpType
AFT = mybir.ActivationFunctionType

B, CIN, H, W = 8, 128, 64, 64
COUT = 256
HW = H * W  # 4096
NT = 32     # position tiles of 128
NK = 9      # taps
TK = NT * NK
MAGIC = 12582912.0  # 1.5 * 2**23: (v + MAGIC) - MAGIC == RNE(v), |v| < 2**22
EPS = 1e-5

GATHER_FP8 = True          # gather source dtype (fp8e3 vs f16)
POOL_TILES = 0             # of 16 tiles per (k, half): last N on Pool engine
GDT = F8E3 if GATHER_FP8 else F16


def build_nc(n_cores: int, dbg: bool = False, use_silu: bool = True):
    nc = bacc.Bacc("TRN2", target_bir_lowering=False, debug=False,
                   num_devices=n_cores, num_swdge_queues=4)

    xpad = nc.dram_tensor("xpad", [128, 66 * 66], F16, kind="ExternalInput")
    # pair image: row p = [ch(p), ch(p+64)]; desc = rows p,p+1 (4 corners)
    xtp = nc.dram_tensor("xtp", [HW, 256], GDT, kind="ExternalInput")
    ow_t = nc.dram_tensor("ow_t", [NK, 128, 18], F16, kind="ExternalInput")
    ob = nc.dram_tensor("ob", [18, 1], F32, kind="ExternalInput")
    dw_t = nc.dram_tensor("dw_t", [NK, 128, COUT], F16, kind="ExternalInput")
    baseY = nc.dram_tensor("baseY", [128, TK], F32, kind="ExternalInput")
    baseX = nc.dram_tensor("baseX", [128, TK], F32, kind="ExternalInput")
    ident = nc.dram_tensor("ident", [128, 128], F16, kind="ExternalInput")
    gamma2 = nc.dram_tensor("gamma2", [128, 2], F32, kind="ExternalInput")
    beta2 = nc.dram_tensor("beta2", [128, 2], F32, kind="ExternalInput")
    yout = nc.dram_tensor("yout", [COUT, HW], F16, kind="ExternalOutput")
    cc_in = nc.dram_tensor("cc_in", [128, 4], F32)
    cc_out = nc.dram_tensor("cc_out", [128, 4], F32)

    dbg_t = {}
    if dbg:
        dbg_t["d_offT"] = nc.dram_tensor("d_offT", [128, NT * 18], F32,
                                         kind="ExternalOutput")
        dbg_t["d_w4"] = nc.dram_tensor("d_w4", [128, TK * 4], F32,
                                       kind="ExternalOutput")
        dbg_t["d_ic"] = nc.dram_tensor("d_ic", [128, TK], I16,
                                       kind="ExternalOutput")
        dbg_t["d_ig"] = nc.dram_tensor("d_ig", [128, NK * 256], I16,
                                       kind="ExternalOutput")
        dbg_t["d_patT"] = nc.dram_tensor("d_patT", [128, NK * 2048], F16,
                                         kind="ExternalOutput")
        dbg_t["d_ysb"] = nc.dram_tensor("d_ysb", [128, 2 * HW], F16,
                                        kind="ExternalOutput")

    with tile.TileContext(nc) as tc:
        _kernel(tc, nc, n_cores, xpad=xpad, xtp=xtp, ow_t=ow_t, ob=ob,
                dw_t=dw_t, baseY=baseY, baseX=baseX, ident=ident,
                gamma2=gamma2, beta2=beta2, yout=yout, cc_in=cc_in,
                cc_out=cc_out, dbg_t=dbg_t, use_silu=use_silu)
    nc.compile()
    return nc


def _coords_half(nc, half, chunk, offT, baseY_sb, baseX_sb, sc, sc2, sc3,
                 w4, icomp):
    """S3 for one 8-tile chunk (72 (t,k) columns): sample coords ->
    4-corner weights w4 and gather index icomp = pyc*64 + pxc.
    y-axis chain on DVE, x-axis chain on Pool."""
    v = nc.vector
    g = nc.gpsimd
    HTK = 8 * NK  # 72
    CO = (2 * half + chunk) * HTK
    ts0 = 16 * half + 8 * chunk
    ts1 = ts0 + 8

    offTv = offT[:].rearrange("p (t c) -> p t c", c=18)
    dy = offTv[:, ts0:ts1, 0:18:2]
    dx = offTv[:, ts0:ts1, 1:18:2]
    byv = baseY_sb[:].rearrange("p (t k) -> p t k", k=NK)[:, ts0:ts1, :]
    bxv = baseX_sb[:].rearrange("p (t k) -> p t k", k=NK)[:, ts0:ts1, :]

    def S(i):
        return sc[:, TK * i + CO: TK * i + CO + HTK]

    def S2(i):
        return sc2[:, TK * i + CO: TK * i + CO + HTK]

    def S3(i):
        return sc3[:, TK * i + CO: TK * i + CO + HTK]

    sy, sx = S(0), S(1)
    y0, x0 = S(2), S(3)
    wy, wx = S(4), S(5)
    ay0, ay1 = S(6), S(7)
    ax0, ax1 = S(8), S(9)
    pyc, pxc = S(10), S(11)

    v.tensor_tensor(sy, dy, byv, AOT.add)
    g.tensor_tensor(sx, dx, bxv, AOT.add)

    def floorv(e, scr, dst, src):
        # dst = RNE(src - 0.5): equals floor(src) except at integer/tie
        # boundaries, where dst = floor +- 1 with frac 1.0 / ~0.0 -- the
        # slot weights then put ~all weight on the true sample point, so
        # the bilinear value error is O(ulp), not discrete.
        # NB: immediates are bf16-rounded at encode; bf16-exact consts only.
        e.tensor_scalar(scr(0), src, -0.5, None, AOT.add)
        e.tensor_scalar(dst, scr(0), MAGIC, MAGIC, AOT.add, AOT.subtract)

    floorv(v, S2, y0, sy)
    floorv(g, S3, x0, sx)
    v.tensor_tensor(wy, sy, y0, AOT.subtract)
    g.tensor_tensor(wx, sx, x0, AOT.subtract)

    def slot_weights(e, scr, w, c0, s0_out, s1_out):
        # s0 = (1-w)*[0<=c0<=62] + w*[c0==-1]
        # s1 = w*[0<=c0<=62] + (1-w)*[c0==63]
        t0, t1, t2, t3 = scr(0), scr(1), scr(2), scr(3)
        e.tensor_scalar(t0, c0, 0.0, None, AOT.is_ge)
        e.tensor_scalar(t1, c0, 62.0, None, AOT.is_le)
        e.tensor_tensor(t0, t0, t1, AOT.mult)             # m0
        e.tensor_scalar(t1, w, -1.0, 1.0, AOT.mult, AOT.add)   # 1-w
        e.tensor_scalar(t2, c0, -1.0, None, AOT.is_equal)      # sL
        e.tensor_scalar(t3, c0, 63.0, None, AOT.is_equal)      # sR
        e.tensor_tensor(t2, w, t2, AOT.mult)              # w*sL
        e.tensor_tensor(t3, t1, t3, AOT.mult)             # (1-w)*sR
        e.tensor_tensor(s0_out, t1, t0, AOT.mult)         # (1-w)*m0
        e.tensor_tensor(s0_out, s0_out, t2, AOT.add)
        e.tensor_tensor(s1_out, w, t0, AOT.mult)          # w*m0
        e.tensor_tensor(s1_out, s1_out, t3, AOT.add)

    slot_weights(v, S2, wy, y0, ay0, ay1)
    slot_weights(g, S3, wx, x0, ax0, ax1)

    v.tensor_scalar(pyc, y0, 0.0, 62.0, AOT.max, AOT.min)
    g.tensor_scalar(pxc, x0, 0.0, 62.0, AOT.max, AOT.min)
    # idx = pyc*64 + pxc -> icomp (int16), cols (k, t) for this half.
    # DVE in half 0 (Pool gates gather issue in the head), Pool in half 1.
    e4 = v if half == 0 else g
    tA = S2(0)
    e4.tensor_scalar(tA, pyc, 64.0, None, AOT.mult)
    e4.tensor_tensor(tA, tA, pxc, AOT.add)
    icv = icomp[:].rearrange("p (k t) -> p k t", t=NT)[:, :, ts0:ts1]
    tAv = tA.rearrange("p (t k) -> p t k", k=NK)
    e4.tensor_copy(icv.rearrange("p k t -> p t k"), tAv)

    # w4 [128, (t, k, corner)], corner order (x-slot, y-slot):
    # 0:(x0,y0) 1:(x0,y1) 2:(x1,y0) 3:(x1,y1)
    w4v = w4[:].rearrange("p (t k c) -> p t k c", k=NK, c=4)[:, ts0:ts1]
    for ci, (ax_, ay_) in enumerate(((ax0, ay0), (ax0, ay1),
                                     (ax1, ay0), (ax1, ay1))):
        axv = ax_.rearrange("p (t k) -> p t k", k=NK)
        ayv = ay_.rearrange("p (t k) -> p t k", k=NK)
        e4.tensor_tensor(w4v[:, :, :, ci], axv, ayv, AOT.mult)


def _igath_half(nc, half, chunk, icomp, ifold, igath):
    """Fold one 8-tile chunk of icomp into the wrapped+replicated gather
    index layout igath[q, (half, k, b, h)] (q in 16, b in 16, h in 8):
    gather j = b*128 + (h*16+q) reads igath[q, (half,k, b*8+h)]; the
    1024-idx gather (half,k,hh) consumes exactly chunk hh's 64 columns."""
    icv = icomp[:].rearrange("p (k t) -> p k t", t=NT)
    iff = ifold[0:16, :].rearrange("q (hf h k b) -> q hf h k b",
                                  hf=2, h=8, k=NK)
    b0 = 8 * chunk
    ts0 = 16 * half + b0
    for h in range(8):
        nc.sync.dma_start(iff[:, half, h, :, b0:b0 + 8],
                          icv[16 * h: 16 * (h + 1), :, ts0: ts0 + 8])
    igf = igath[:].rearrange("q (hf k b h) -> q hf k b h", hf=2, k=NK, h=8)
    nc.vector.tensor_copy(
        igf[0:16, half, :, b0:b0 + 8, :],
        iff[:, half, :, :, b0:b0 + 8].rearrange("q h k b -> q k b h"))
    for rep in range(1, 8):
        nc.sync.dma_start(igf[16 * rep: 16 * (rep + 1), half, :,
                              b0:b0 + 8, :],
                          igf[0:16, half, :, b0:b0 + 8, :])


def _kernel(tc, nc, n_cores, *, xpad, xtp, ow_t, ob, dw_t, baseY, baseX,
            ident, gamma2, beta2, yout, cc_in, cc_out, dbg_t=None,
            use_silu=True):
    from contextlib import ExitStack
    ctx = ExitStack()
    with ctx:
        pool = ctx.enter_context(tc.tile_pool(name="main", bufs=1))
        gtp = ctx.enter_context(tc.tile_pool(name="gt", bufs=3))
        ppp = ctx.enter_context(tc.tile_pool(name="pp", bufs=8))
        patp = ctx.enter_context(tc.tile_pool(name="patp", bufs=2))
        ps_tr = ctx.enter_context(
            tc.tile_pool(name="ps_tr", bufs=3, space="PSUM"))
        actx = ExitStack()
        ps_off = actx.enter_context(
            tc.tile_pool(name="ps_off", bufs=2, space="PSUM"))
        ps_offT = actx.enter_context(
            tc.tile_pool(name="ps_offT", bufs=1, space="PSUM"))

        v = nc.vector
        s = nc.scalar
        g = nc.gpsimd

        # ---- constants / weights ----
        # xpad lives in the gather-tile ring: it is dead after the offset
        # conv, so its slot recycles for gt tiles.
        xpad_sb = gtp.tile([128, 66 * 66], F16, tag="gt")
        nc.sync.dma_start(xpad_sb[:], xpad.ap())
        ow_sb = pool.tile([128, NK * 18], F16)    # [c, (k, o)]
        nc.sync.dma_start(
            ow_sb[:].rearrange("c (k o) -> c k o", k=NK),
            ow_t.ap().rearrange("k c o -> c k o"))
        ob_sb = pool.tile([18, 1], F32)
        nc.sync.dma_start(ob_sb[:], ob.ap())
        dw_sb = pool.tile([128, NK * COUT], F16)  # [c, (k, o)]
        nc.sync.dma_start(
            dw_sb[:].rearrange("c (k o) -> c k o", k=NK),
            dw_t.ap().rearrange("k c o -> c k o"))
        baseY_sb = pool.tile([128, TK], F32)
        nc.sync.dma_start(baseY_sb[:], baseY.ap())
        baseX_sb = pool.tile([128, TK], F32)
        nc.sync.dma_start(baseX_sb[:], baseX.ap())
        ident_sb = pool.tile([128, 128], F16)
        nc.sync.dma_start(ident_sb[:], ident.ap())
        gamma_sb = pool.tile([128, 2], F32)
        nc.sync.dma_start(gamma_sb[:], gamma2.ap())
        beta_sb = pool.tile([128, 2], F32)
        nc.sync.dma_start(beta_sb[:], beta2.ap())

        # ---- persistent tiles ----
        offC = pool.tile([18, HW], F16)
        xpv = xpad_sb[:].rearrange("p (a b) -> p a b", a=66)
        offT = pool.tile([128, NT * 18], F32)
        sc = pool.tile([128, 12 * TK], F32)
        sc2 = pool.tile([128, 4 * TK], F32)
        sc3 = pool.tile([128, 4 * TK], F32)
        w4 = pool.tile([128, TK * 4], F32)
        icomp = pool.tile([128, TK], I16)   # idx per (k, t)
        ifold = pool.tile([16, 2 * 8 * NK * 16], I16)
        igath = pool.tile([128, 2 * NK * 128], I16)
        ysb = pool.tile([128, 2 * HW], F16)
        stats = pool.tile([128, 32], F32)
        sq_scr = pool.tile([128, 512], F16)

        # gather source: xtp rows (p, p+1) -> 512 elements = 4 corners
        xtp_pairs = bass.AP(tensor=xtp, offset=0, ap=[[256, HW - 1], [1, 512]])

        # ---- phase A: S1/S2/S3/S4 for BOTH halves up front, so the PE
        # stream never blocks half-1 coords behind half-0 matmuls ----
        for half in range(2):
            for nl in range(4):
                # S1: offset conv for n-group nl (bias via evac)
                n = 4 * half + nl
                po = ps_off.tile([18, 512], F32, tag="ps_off")
                for k in range(NK):
                    ky, kx = k // 3, k % 3
                    rhs = xpv[:, 8 * n + ky: 8 * n + ky + 8, kx: kx + 64]
                    nc.tensor.matmul(po[:], ow_sb[:, 18 * k: 18 * (k + 1)],
                                     rhs, start=(k == 0), stop=(k == NK - 1))
                s.activation(offC[:, 512 * n: 512 * (n + 1)], po[:],
                             AFT.Identity, bias=ob_sb[:, 0:1])
                # S2: transposes for this n-group's 4 tiles
                t0 = n * 4
                pt = ps_offT.tile([128, 4, 18], F16, tag="ps_offT")
                for ti in range(4):
                    nc.tensor.transpose(
                        pt[:, ti, :],
                        offC[:, 128 * (t0 + ti): 128 * (t0 + ti + 1)],
                        ident_sb[0:18, 0:18])
                v.tensor_copy(offT[:, 18 * t0: 18 * (t0 + 4)], pt[:])
                # S3+S4 per 8-tile chunk as soon as its offT is ready
                if nl % 2 == 1:
                    chunk = nl // 2
                    _coords_half(nc, half, chunk, offT, baseY_sb, baseX_sb,
                                 sc, sc2, sc3, w4, icomp)
                    _igath_half(nc, half, chunk, icomp, ifold, igath)

        actx.close()
        ps_y = ctx.enter_context(
            tc.tile_pool(name="ps_y", bufs=5, space="PSUM"))

        # ---- phase B: per (half, tap): gather -> diag-build -> 4
        # accumulating diag matmuls (blend + transpose in one PE pass,
        # reading fp8 directly) -> evac. Half-0 main matmuls interleave
        # into half-1's tap loop to keep the PE stream dense. ----
        w4v = w4[:].rearrange("p (t k c) -> p t k c", k=NK, c=4)
        patTs = []
        gseq = 0

        def mm_group(half, patT, gi):
            n, M = gi // 2, gi % 2
            py_ = ps_y.tile([128, 512], F32, tag="ps_y")
            for k in range(NK):
                nc.tensor.matmul(
                    py_[:],
                    dw_sb[:, COUT * k + 128 * M: COUT * k + 128 * (M + 1)],
                    patT[:, 2048 * k + 512 * n: 2048 * k + 512 * (n + 1)],
                    start=(k == 0), stop=(k == NK - 1))
            ncol = half * 4 + n
            dst = ysb[:, HW * M + 512 * ncol: HW * M + 512 * (ncol + 1)]
            s.activation(dst, py_[:], AFT.Copy, bias=0.0,
                         accum_out=stats[:, 8 * M + ncol: 8 * M + ncol + 1])
            if (half * 8 + gi) % 2 == 0:
                v.scalar_tensor_tensor(
                    sq_scr[:], dst, 1.0, dst, AOT.mult, AOT.mult,
                    accum_out=stats[:, 16 + 8 * M + ncol:
                                    16 + 8 * M + ncol + 1])
            else:
                s.activation(sq_scr[:], py_[:], AFT.Square,
                             accum_out=stats[:, 16 + 8 * M + ncol:
                                             16 + 8 * M + ncol + 1])

        def issue_gather(half, k, qn):
            # 2 x 1024-idx calls: a single 2048-idx SWDGE gather crashes HW
            gt = gtp.tile([128, 16, 512], GDT, tag="gt")
            base = (half * NK + k) * 128
            for hh in range(2):
                g.dma_gather(gt[:, 8 * hh: 8 * (hh + 1), :], xtp_pairs,
                             igath[:, base + 64 * hh: base + 64 * (hh + 1)],
                             1024, 1024, 512, elem_step=256,
                             queue_num=0)
            return gt

        seq = [(h, k) for h in range(2) for k in range(NK)]
        gts = {0: issue_gather(*seq[0], 0)}
        for i, (half, k) in enumerate(seq):
            if k == 0:
                patT = patp.tile([128, NK * 2048], F16, tag="patT")
                patTs.append(patT)
            if i + 1 < len(seq):
                gts[i + 1] = issue_gather(*seq[i + 1], i + 1)
            gt = gts.pop(i)
            if True:
                for tq in range(4):       # groups of 4 tiles per PSUM bank
                    ptr = ps_tr.tile([128, 512], F32, tag="ptr")
                    for ti in range(4):
                        tl = 4 * tq + ti
                        t = half * 16 + tl
                        wsl = w4v[:, t, k, :]
                        deng = s if tl == 6 else v
                        D = ppp.tile([128, 4, 128], F16, tag="pp")
                        for ci in range(4):
                            if deng is s:
                                s.activation(D[:, ci, :], ident_sb[:],
                                             AFT.Copy, bias=0.0,
                                             scale=wsl[:, ci: ci + 1])
                            else:
                                deng.tensor_scalar(D[:, ci, :], ident_sb[:],
                                                   wsl[:, ci: ci + 1], None,
                                                   AOT.mult)
                        for ci in range(4):
                            nc.tensor.matmul(
                                ptr[:, 128 * ti: 128 * (ti + 1)],
                                gt[:, tl, 128 * ci: 128 * (ci + 1)],
                                D[:, ci, :],
                                start=(ci == 0), stop=(ci == 3))
                    s.activation(
                        patT[:, 2048 * k + 512 * tq:
                             2048 * k + 512 * tq + 512],
                        ptr[:], AFT.Copy, bias=0.0)
                if half == 1 and k >= 1:
                    mm_group(0, patTs[0], k - 1)
                if half == 1 and k == 7:
                    # pre-accumulate taps 0..7 for half-1 groups 0..3 while
                    # tap 8 is still in flight
                    partials = []
                    for gi in range(4):
                        n, M = gi // 2, gi % 2
                        py_ = ps_y.tile([128, 512], F32, tag="ps_y")
                        partials.append(py_)
                        for kk in range(8):
                            nc.tensor.matmul(
                                py_[:],
                                dw_sb[:, COUT * kk + 128 * M:
                                      COUT * kk + 128 * (M + 1)],
                                patT[:, 2048 * kk + 512 * n:
                                     2048 * kk + 512 * (n + 1)],
                                start=(kk == 0), stop=False)
        for gi in range(4):
            n, M = gi // 2, gi % 2
            py_ = partials[gi]
            nc.tensor.matmul(
                py_[:],
                dw_sb[:, COUT * 8 + 128 * M: COUT * 8 + 128 * (M + 1)],
                patTs[1][:, 2048 * 8 + 512 * n: 2048 * 8 + 512 * (n + 1)],
                start=False, stop=True)
            ncol = 4 + n
            dst = ysb[:, HW * M + 512 * ncol: HW * M + 512 * (ncol + 1)]
            s.activation(dst, py_[:], AFT.Copy, bias=0.0,
                         accum_out=stats[:, 8 * M + ncol: 8 * M + ncol + 1])
            v.scalar_tensor_tensor(
                sq_scr[:], dst, 1.0, dst, AOT.mult, AOT.mult,
                accum_out=stats[:, 16 + 8 * M + ncol: 16 + 8 * M + ncol + 1])
        for gi in range(4, 8):
            mm_group(1, patTs[1], gi)

            if dbg_t and half == 1:
                nc.sync.dma_start(dbg_t["d_patT"].ap(), patT[:])

        if dbg_t:
            nc.sync.dma_start(dbg_t["d_offT"].ap(), offT[:])
            nc.sync.dma_start(dbg_t["d_w4"].ap(), w4[:])
            nc.sync.dma_start(dbg_t["d_ic"].ap(), icomp[:])
            nc.sync.dma_start(dbg_t["d_ig"].ap(), igath[0:128, :])
            nc.sync.dma_start(dbg_t["d_ysb"].ap(), ysb[:])

        # ---- S10: stats -> allreduce -> scale/shift ----
        st4 = pool.tile([128, 4], F32)
        stv = stats[:].rearrange("p (a n) -> p a n", n=8)
        for a in range(4):
            v.tensor_reduce(st4[:, a:a + 1], stv[:, a, :],
                            mybir.AxisListType.X, AOT.add)

        if n_cores > 1:
            nc.sync.dma_start(cc_in.ap(), st4[:])
            g.collective_compute(
                "AllReduce", AOT.add, replica_groups=[list(range(n_cores))],
                ins=[cc_in.ap()], outs=[cc_out.ap()])
            nc.sync.dma_start(st4[:], cc_out.ap())

        NTOT = float(n_cores * HW)
        mean2 = pool.tile([128, 2], F32)
        var2 = pool.tile([128, 2], F32)
        rstd2 = pool.tile([128, 2], F32)
        nsc = pool.tile([128, 2], F32)
        v.tensor_scalar(mean2[:], st4[:, 0:2], 1.0 / NTOT, None, AOT.mult)
        v.tensor_scalar(var2[:], st4[:, 2:4], 1.0 / NTOT, None, AOT.mult)
        v.tensor_tensor(rstd2[:], mean2[:], mean2[:], AOT.mult)
        v.tensor_tensor(var2[:], var2[:], rstd2[:], AOT.subtract)
        v.tensor_scalar(var2[:], var2[:], EPS, None, AOT.add)
        # rstd = 1/sqrt(var) via Newton (avoids act-table reload for Sqrt):
        # x <- x*(1.5 - 0.5*v*x^2), seed 0.25 converges for var in (0, 48)
        v.memset(rstd2[:], 0.25)
        for _ in range(6):
            v.tensor_tensor(nsc[:], rstd2[:], rstd2[:], AOT.mult)
            v.tensor_tensor(nsc[:], nsc[:], var2[:], AOT.mult)
            v.tensor_scalar(nsc[:], nsc[:], -0.5, 1.5, AOT.mult, AOT.add)
            v.tensor_tensor(rstd2[:], rstd2[:], nsc[:], AOT.mult)
        scl = pool.tile([128, 2], F32)
        sft = pool.tile([128, 2], F32)
        v.tensor_tensor(scl[:], gamma_sb[:], rstd2[:], AOT.mult)
        v.tensor_tensor(sft[:], mean2[:], scl[:], AOT.mult)
        v.tensor_tensor(sft[:], beta_sb[:], sft[:], AOT.subtract)

        # ---- S11: normalize + SiLU + output ----
        CH = HW // 2
        for M in range(2):
            yfin = pool.tile([128, HW], F16, tag="yfin")
            for c_ in range(2):
                sl = slice(CH * c_, CH * (c_ + 1))
                ysl = ysb[:, HW * M + CH * c_: HW * M + CH * (c_ + 1)]
                if use_silu:
                    s.activation(yfin[:, sl], ysl, AFT.Silu,
                                 bias=sft[:, M:M + 1], scale=scl[:, M:M + 1])
                else:  # CoreSim has no Silu; z * sigmoid(z) fallback
                    zsc = gtp.tile([128, HW], F16, tag="gt")
                    v.tensor_scalar(zsc[:, sl], ysl, scl[:, M:M + 1],
                                    sft[:, M:M + 1], AOT.mult, AOT.add)
                    s.activation(yfin[:, sl], zsc[:, sl], AFT.Sigmoid,
                                 bias=0.0)
                    v.tensor_tensor(yfin[:, sl], zsc[:, sl], yfin[:, sl],
                                    AOT.mult)
                (nc.sync if (2 * M + c_) % 2 == 0 else nc.scalar).dma_start(
                    bass.AP(tensor=yout, offset=M * 128 * HW + CH * c_,
                            ap=[[HW, 128], [1, CH]]),
                    yfin[:, sl])


# =========================================================
# host side
# =========================================================
_NC_CACHE = {}


def _get_nc(n_cores):
    if n_cores not in _NC_CACHE:
        _NC_CACHE[n_cores] = build_nc(n_cores)
    return _NC_CACHE[n_cores]


def make_in_maps(x, offset_w, offset_b, dconv_w, dconv_b, bn_gamma, bn_beta,
                 n_cores=8):
    x = np.asarray(x, np.float32)
    ow = np.asarray(offset_w, np.float32)
    dw = np.asarray(dconv_w, np.float32)
    ow_t = np.ascontiguousarray(
        ow.reshape(18, 128, 9).transpose(2, 1, 0)).astype(np.float16)
    dw_t = np.ascontiguousarray(
        dw.reshape(COUT, 128, 9).transpose(2, 1, 0)).astype(np.float16)
    ob = np.asarray(offset_b, np.float32).reshape(18, 1).copy()
    p = np.arange(128)
    t = np.arange(NT)
    k = np.arange(NK)
    ky, kx = k // 3, k % 3
    baseY = ((t[None, :, None] * 2 + (p[:, None, None] // 64)) - 1
             + ky[None, None, :]).reshape(128, TK).astype(np.float32)
    baseX = (((p[:, None, None] % 64)) - 1
             + kx[None, None, :] + 0 * t[None, :, None]).reshape(
                 128, TK).astype(np.float32)
    baseY = np.ascontiguousarray(baseY)
    baseX = np.ascontiguousarray(baseX)
    ident = np.eye(128, dtype=np.float16)
    gamma2 = np.ascontiguousarray(
        np.asarray(bn_gamma, np.float32).reshape(2, 128).T)
    beta2 = np.ascontiguousarray(
        np.asarray(bn_beta, np.float32).reshape(2, 128).T)
    gnp = mybir.dt.np(GDT)

    in_maps = []
    for c in range(n_cores):
        xb = x[c]
        xp = np.zeros((128, 66, 66), np.float16)
        xp[:, 1:65, 1:65] = xb.astype(np.float16)
        xT = np.ascontiguousarray(xb.reshape(128, HW).T)  # [pos, ch] f32
        xtp = np.zeros((HW, 256), np.float32)
        xtp[:, 0:128] = xT
        xtp[:-64, 128:256] = xT[64:]
        in_maps.append({
            "xpad": np.ascontiguousarray(xp.reshape(128, 66 * 66)),
            "xtp": xtp.astype(gnp),
            "ow_t": ow_t, "ob": ob, "dw_t": dw_t,
            "baseY": baseY, "baseX": baseX, "ident": ident,
            "gamma2": gamma2, "beta2": beta2,
        })
    return in_maps


def kernel(x, offset_w, offset_b, dconv_w, dconv_b, bn_gamma, bn_beta,
           trace=False):
    n_cores = 8
    nc = _get_nc(n_cores)
    in_maps = make_in_maps(x, offset_w, offset_b, dconv_w, dconv_b,
                           bn_gamma, bn_beta, n_cores)
    res = run_bass_kernel_spmd(nc, in_maps, list(range(n_cores)), trace=trace)
    out = np.stack([res.results[c]["yout"].reshape(COUT, H, W)
                    for c in range(n_cores)])
    kernel.last_result = res
    return out.astype(np.float32)


# revision 64
# speedup vs baseline: 1.5665x; 1.0024x over previous
# Deformable conv2d (offset conv -> bilinear sampling -> conv -> BN -> SiLU)
# on 8 trn2 NeuronCores, data-parallel over batch (1 image per core).
#
# v2: single 512B gather descriptor per (tap, position) fetching all 4
# bilinear corners from an fp8e3 "pair" image layout xtp[p] =
# [ch(p), ch(p+64)]; a descriptor covers rows p, p+1 = corners
# (y0,x0),(y1,x0),(y0,x1),(y1,x1). Blend = scalar_tensor_tensor chains
# spread across DVE / Act / Pool engines.
import sys

for _p in ("/opt/trn_rl_repo",):
    if _p not in sys.path:
        sys.path.insert(0, _p)

import numpy as np

import concourse.bacc as bacc
import concourse.bass as bass
import concourse.mybir as mybir
import concourse.tile as tile
from concourse.bass_utils import run_bass_kernel_spmd

F32 = mybir.dt.float32
F16 = mybir.dt.float16
F8E3 = mybir.dt.float8e3
I16 = mybir.dt.int16
AOT = mybir.AluOpType
AFT = mybir.ActivationFunctionType

B, CIN, H, W = 8, 128, 64, 64
COUT = 256
HW = H * W  # 4096
NT = 32     # position tiles of 128
NK = 9      # taps
TK = NT * NK
MAGIC = 12582912.0  # 1.5 * 2**23: (v + MAGIC) - MAGIC == RNE(v), |v| < 2**22
EPS = 1e-5

GATHER_FP8 = True          # gather source dtype (fp8e3 vs f16)
POOL_TILES = 0             # of 16 tiles per (k, half): last N on Pool engine
GDT = F8E3 if GATHER_FP8 else F16


def build_nc(n_cores: int, dbg: bool = False, use_silu: bool = True):
    nc = bacc.Bacc("TRN2", target_bir_lowering=False, debug=False,
                   num_devices=n_cores, num_swdge_queues=4)

    xpad = nc.dram_tensor("xpad", [128, 66 * 66], F16, kind="ExternalInput")
    # pair image: row p = [ch(p), ch(p+64)]; desc = rows p,p+1 (4 corners)
    xtp = nc.dram_tensor("xtp", [HW, 256], GDT, kind="ExternalInput")
    ow_t = nc.dram_tensor("ow_t", [NK, 128, 18], F16, kind="ExternalInput")
    ob = nc.dram_tensor("ob", [18, 1], F32, kind="ExternalInput")
    dw_t = nc.dram_tensor("dw_t", [NK, 128, COUT], F16, kind="ExternalInput")
    baseY = nc.dram_tensor("baseY", [128, TK], F32, kind="ExternalInput")
    baseX = nc.dram_tensor("baseX", [128, TK], F32, kind="ExternalInput")
    ident = nc.dram_tensor("ident", [128, 128], F16, kind="ExternalInput")
    gamma2 = nc.dram_tensor("gamma2", [128, 2], F32, kind="ExternalInput")
    beta2 = nc.dram_tensor("beta2", [128, 2], F32, kind="ExternalInput")
    yout = nc.dram_tensor("yout", [COUT, HW], F16, kind="ExternalOutput")
    cc_in = nc.dram_tensor("cc_in", [128, 4], F32)
    cc_out = nc.dram_tensor("cc_out", [128, 4], F32)

    dbg_t = {}
    if dbg:
        dbg_t["d_offT"] = nc.dram_tensor("d_offT", [128, NT * 18], F32,
                                         kind="ExternalOutput")
        dbg_t["d_w4"] = nc.dram_tensor("d_w4", [128, TK * 4], F32,
                                       kind="ExternalOutput")
        dbg_t["d_ic"] = nc.dram_tensor("d_ic", [128, TK], I16,
                                       kind="ExternalOutput")
        dbg_t["d_ig"] = nc.dram_tensor("d_ig", [128, NK * 256], I16,
                                       kind="ExternalOutput")
        dbg_t["d_patT"] = nc.dram_tensor("d_patT", [128, NK * 2048], F16,
                                         kind="ExternalOutput")
        dbg_t["d_ysb"] = nc.dram_tensor("d_ysb", [128, 2 * HW], F16,
                                        kind="ExternalOutput")

    with tile.TileContext(nc) as tc:
        _kernel(tc, nc, n_cores, xpad=xpad, xtp=xtp, ow_t=ow_t, ob=ob,
                dw_t=dw_t, baseY=baseY, baseX=baseX, ident=ident,
                gamma2=gamma2, beta2=beta2, yout=yout, cc_in=cc_in,
                cc_out=cc_out, dbg_t=dbg_t, use_silu=use_silu)
    nc.compile()
    return nc


def _coords_half(nc, half, chunk, offT, baseY_sb, baseX_sb, sc, sc2, sc3,
                 w4, icomp):
    """S3 for one 8-tile chunk (72 (t,k) columns): sample coords ->
    4-corner weights w4 and gather index icomp = pyc*64 + pxc.
    y-axis chain on DVE, x-axis chain on Pool."""
    v = nc.vector
    g = nc.gpsimd
    HTK = 8 * NK  # 72
    CO = (2 * half + chunk) * HTK
    ts0 = 16 * half + 8 * chunk
    ts1 = ts0 + 8

    offTv = offT[:].rearrange("p (t c) -> p t c", c=18)
    dy = offTv[:, ts0:ts1, 0:18:2]
    dx = offTv[:, ts0:ts1, 1:18:2]
    byv = baseY_sb[:].rearrange("p (t k) -> p t k", k=NK)[:, ts0:ts1, :]
    bxv = baseX_sb[:].rearrange("p (t k) -> p t k", k=NK)[:, ts0:ts1, :]

    def S(i):
        return sc[:, TK * i + CO: TK * i + CO + HTK]

    def S2(i):
        return sc2[:, TK * i + CO: TK * i + CO + HTK]

    def S3(i):
        return sc3[:, TK * i + CO: TK * i + CO + HTK]

    sy, sx = S(0), S(1)
    y0, x0 = S(2), S(3)
    wy, wx = S(4), S(5)
    ay0, ay1 = S(6), S(7)
    ax0, ax1 = S(8), S(9)
    pyc, pxc = S(10), S(11)

    v.tensor_tensor(sy, dy, byv, AOT.add)
    g.tensor_tensor(sx, dx, bxv, AOT.add)

    def floorv(e, scr, dst, src):
        # dst = RNE(src - 0.5): equals floor(src) except at integer/tie
        # boundaries, where dst = floor +- 1 with frac 1.0 / ~0.0 -- the
        # slot weights then put ~all weight on the true sample point, so
        # the bilinear value error is O(ulp), not discrete.
        # NB: immediates are bf16-rounded at encode; bf16-exact consts only.
        e.tensor_scalar(scr(0), src, -0.5, None, AOT.add)
        e.tensor_scalar(dst, scr(0), MAGIC, MAGIC, AOT.add, AOT.subtract)

    floorv(v, S2, y0, sy)
    floorv(g, S3, x0, sx)
    v.tensor_tensor(wy, sy, y0, AOT.subtract)
    g.tensor_tensor(wx, sx, x0, AOT.subtract)

    def slot_weights(e, scr, w, c0, s0_out, s1_out):
        # s0 = (1-w)*[0<=c0<=62] + w*[c0==-1]
        # s1 = w*[0<=c0<=62] + (1-w)*[c0==63]
        t0, t1, t2, t3 = scr(0), scr(1), scr(2), scr(3)
        e.tensor_scalar(t0, c0, 0.0, None, AOT.is_ge)
        e.tensor_scalar(t1, c0, 62.0, None, AOT.is_le)
        e.tensor_tensor(t0, t0, t1, AOT.mult)             # m0
        e.tensor_scalar(t1, w, -1.0, 1.0, AOT.mult, AOT.add)   # 1-w
        e.tensor_scalar(t2, c0, -1.0, None, AOT.is_equal)      # sL
        e.tensor_scalar(t3, c0, 63.0, None, AOT.is_equal)      # sR
        e.tensor_tensor(t2, w, t2, AOT.mult)              # w*sL
        e.tensor_tensor(t3, t1, t3, AOT.mult)             # (1-w)*sR
        e.tensor_tensor(s0_out, t1, t0, AOT.mult)         # (1-w)*m0
        e.tensor_tensor(s0_out, s0_out, t2, AOT.add)
        e.tensor_tensor(s1_out, w, t0, AOT.mult)          # w*m0
        e.tensor_tensor(s1_out, s1_out, t3, AOT.add)

    slot_weights(v, S2, wy, y0, ay0, ay1)
    slot_weights(g, S3, wx, x0, ax0, ax1)

    v.tensor_scalar(pyc, y0, 0.0, 62.0, AOT.max, AOT.min)
    g.tensor_scalar(pxc, x0, 0.0, 62.0, AOT.max, AOT.min)
    # idx = pyc*64 + pxc -> icomp (int16), cols (k, t) for this half.
    # DVE in half 0 (Pool gates gather issue in the head), Pool in half 1.
    e4 = v if half == 0 else g
    tA = S2(0)
    e4.tensor_scalar(tA, pyc, 64.0, None, AOT.mult)
    e4.tensor_tensor(tA, tA, pxc, AOT.add)
    icv = icomp[:].rearrange("p (k t) -> p k t", t=NT)[:, :, ts0:ts1]
    tAv = tA.rearrange("p (t k) -> p t k", k=NK)
    e4.tensor_copy(icv.rearrange("p k t -> p t k"), tAv)

    # w4 [128, (t, k, corner)], corner order (x-slot, y-slot):
    # 0:(x0,y0) 1:(x0,y1) 2:(x1,y0) 3:(x1,y1)
    w4v = w4[:].rearrange("p (t k c) -> p t k c", k=NK, c=4)[:, ts0:ts1]
    for ci, (ax_, ay_) in enumerate(((ax0, ay0), (ax0, ay1),
                                     (ax1, ay0), (ax1, ay1))):
        axv = ax_.rearrange("p (t k) -> p t k", k=NK)
        ayv = ay_.rearrange("p (t k) -> p t k", k=NK)
        e4.tensor_tensor(w4v[:, :, :, ci], axv, ayv, AOT.mult)


def _igath_half(nc, half, chunk, icomp, ifold, igath):
    """Fold one 8-tile chunk of icomp into the wrapped+replicated gather
    index layout igath[q, (half, k, b, h)] (q in 16, b in 16, h in 8):
    gather j = b*128 + (h*16+q) reads igath[q, (half,k, b*8+h)]; the
    1024-idx gather (half,k,hh) consumes exactly chunk hh's 64 columns."""
    icv = icomp[:].rearrange("p (k t) -> p k t", t=NT)
    iff = ifold[0:16, :].rearrange("q (hf h k b) -> q hf h k b",
                                  hf=2, h=8, k=NK)
    b0 = 8 * chunk
    ts0 = 16 * half + b0
    for h in range(8):
        nc.sync.dma_start(iff[:, half, h, :, b0:b0 + 8],
                          icv[16 * h: 16 * (h + 1), :, ts0: ts0 + 8])
    igf = igath[:].rearrange("q (hf k b h) -> q hf k b h", hf=2, k=NK, h=8)
    nc.vector.tensor_copy(
        igf[0:16, half, :, b0:b0 + 8, :],
        iff[:, half, :, :, b0:b0 + 8].rearrange("q h k b -> q k b h"))
    for rep in range(1, 8):
        nc.sync.dma_start(igf[16 * rep: 16 * (rep + 1), half, :,
                              b0:b0 + 8, :],
                          igf[0:16, half, :, b0:b0 + 8, :])


def _kernel(tc, nc, n_cores, *, xpad, xtp, ow_t, ob, dw_t, baseY, baseX,
            ident, gamma2, beta2, yout, cc_in, cc_out, dbg_t=None,
            use_silu=True):
    from contextlib import ExitStack
    ctx = ExitStack()
    with ctx:
        pool = ctx.enter_context(tc.tile_pool(name="main", bufs=1))
        gtp = ctx.enter_context(tc.tile_pool(name="gt", bufs=3))
        ppp = ctx.enter_context(tc.tile_pool(name="pp", bufs=8))
        patp = ctx.enter_context(tc.tile_pool(name="patp", bufs=2))
        ps_tr = ctx.enter_context(
            tc.tile_pool(name="ps_tr", bufs=3, space="PSUM"))
        actx = ExitStack()
        ps_off = actx.enter_context(
            tc.tile_pool(name="ps_off", bufs=3, space="PSUM"))
        ps_offT = actx.enter_context(
            tc.tile_pool(name="ps_offT", bufs=2, space="PSUM"))

        v = nc.vector
        s = nc.scalar
        g = nc.gpsimd

        # ---- constants / weights ----
        # xpad lives in the gather-tile ring: it is dead after the offset
        # conv, so its slot recycles for gt tiles.
        xpad_sb = gtp.tile([128, 66 * 66], F16, tag="gt")
        nc.sync.dma_start(xpad_sb[:], xpad.ap())
        ow_sb = pool.tile([128, NK * 18], F16)    # [c, (k, o)]
        nc.sync.dma_start(
            ow_sb[:].rearrange("c (k o) -> c k o", k=NK),
            ow_t.ap().rearrange("k c o -> c k o"))
        ob_sb = pool.tile([18, 1], F32)
        nc.sync.dma_start(ob_sb[:], ob.ap())
        dw_sb = pool.tile([128, NK * COUT], F16)  # [c, (k, o)]
        nc.sync.dma_start(
            dw_sb[:].rearrange("c (k o) -> c k o", k=NK),
            dw_t.ap().rearrange("k c o -> c k o"))
        baseY_sb = pool.tile([128, TK], F32)
        nc.sync.dma_start(baseY_sb[:], baseY.ap())
        baseX_sb = pool.tile([128, TK], F32)
        nc.sync.dma_start(baseX_sb[:], baseX.ap())
        ident_sb = pool.tile([128, 128], F16)
        nc.sync.dma_start(ident_sb[:], ident.ap())
        gamma_sb = pool.tile([128, 2], F32)
        nc.sync.dma_start(gamma_sb[:], gamma2.ap())
        beta_sb = pool.tile([128, 2], F32)
        nc.sync.dma_start(beta_sb[:], beta2.ap())

        # ---- persistent tiles ----
        offC = pool.tile([18, HW], F16)
        xpv = xpad_sb[:].rearrange("p (a b) -> p a b", a=66)
        offT = pool.tile([128, NT * 18], F32)
        sc = pool.tile([128, 12 * TK], F32)
        sc2 = pool.tile([128, 4 * TK], F32)
        sc3 = pool.tile([128, 4 * TK], F32)
        w4 = pool.tile([128, TK * 4], F32)
        icomp = pool.tile([128, TK], I16)   # idx per (k, t)
        ifold = pool.tile([16, 2 * 8 * NK * 16], I16)
        igath = pool.tile([128, 2 * NK * 128], I16)
        ysb = pool.tile([128, 2 * HW], F16)
        stats = pool.tile([128, 32], F32)
        sq_scr = pool.tile([128, 512], F16)

        # gather source: xtp rows (p, p+1) -> 512 elements = 4 corners
        xtp_pairs = bass.AP(tensor=xtp, offset=0, ap=[[256, HW - 1], [1, 512]])

        # ---- phase A: S1/S2/S3/S4 for BOTH halves up front, so the PE
        # stream never blocks half-1 coords behind half-0 matmuls ----
        for half in range(2):
            for nl in range(4):
                # S1: offset conv for n-group nl (bias via evac)
                n = 4 * half + nl
                po = ps_off.tile([18, 512], F32, tag="ps_off")
                for k in range(NK):
                    ky, kx = k // 3, k % 3
                    rhs = xpv[:, 8 * n + ky: 8 * n + ky + 8, kx: kx + 64]
                    nc.tensor.matmul(po[:], ow_sb[:, 18 * k: 18 * (k + 1)],
                                     rhs, start=(k == 0), stop=(k == NK - 1))
                s.activation(offC[:, 512 * n: 512 * (n + 1)], po[:],
                             AFT.Identity, bias=ob_sb[:, 0:1])
                # S2: transposes for this n-group's 4 tiles
                t0 = n * 4
                pt = ps_offT.tile([128, 4, 18], F16, tag="ps_offT")
                for ti in range(4):
                    nc.tensor.transpose(
                        pt[:, ti, :],
                        offC[:, 128 * (t0 + ti): 128 * (t0 + ti + 1)],
                        ident_sb[0:18, 0:18])
                v.tensor_copy(offT[:, 18 * t0: 18 * (t0 + 4)], pt[:])
                # S3+S4 per 8-tile chunk as soon as its offT is ready
                if nl % 2 == 1:
                    chunk = nl // 2
                    _coords_half(nc, half, chunk, offT, baseY_sb, baseX_sb,
                                 sc, sc2, sc3, w4, icomp)
                    _igath_half(nc, half, chunk, icomp, ifold, igath)

        actx.close()
        ps_y = ctx.enter_context(
            tc.tile_pool(name="ps_y", bufs=5, space="PSUM"))

        # ---- phase B: per (half, tap): gather -> diag-build -> 4
        # accumulating diag matmuls (blend + transpose in one PE pass,
        # reading fp8 directly) -> evac. Half-0 main matmuls interleave
        # into half-1's tap loop to keep the PE stream dense. ----
        w4v = w4[:].rearrange("p (t k c) -> p t k c", k=NK, c=4)
        patTs = []
        gseq = 0

        def mm_group(half, patT, gi):
            n, M = gi // 2, gi % 2
            py_ = ps_y.tile([128, 512], F32, tag="ps_y")
            for k in range(NK):
                nc.tensor.matmul(
                    py_[:],
                    dw_sb[:, COUT * k + 128 * M: COUT * k + 128 * (M + 1)],
                    patT[:, 2048 * k + 512 * n: 2048 * k + 512 * (n + 1)],
                    start=(k == 0), stop=(k == NK - 1))
            ncol = half * 4 + n
            dst = ysb[:, HW * M + 512 * ncol: HW * M + 512 * (ncol + 1)]
            s.activation(dst, py_[:], AFT.Copy, bias=0.0,
                         accum_out=stats[:, 8 * M + ncol: 8 * M + ncol + 1])
            if (half * 8 + gi) % 2 == 0:
                v.scalar_tensor_tensor(
                    sq_scr[:], dst, 1.0, dst, AOT.mult, AOT.mult,
                    accum_out=stats[:, 16 + 8 * M + ncol:
                                    16 + 8 * M + ncol + 1])
            else:
                s.activation(sq_scr[:], py_[:], AFT.Square,
                             accum_out=stats[:, 16 + 8 * M + ncol:
                                             16 + 8 * M + ncol + 1])

        def issue_gather(half, k, qn):
            # 2 x 1024-idx calls: a single 2048-idx SWDGE gather crashes HW
            gt = gtp.tile([128, 16, 512], GDT, tag="gt")
            base = (half * NK + k) * 128
            for hh in range(2):
                g.dma_gather(gt[:, 8 * hh: 8 * (hh + 1), :], xtp_pairs,
                             igath[:, base + 64 * hh: base + 64 * (hh + 1)],
                             1024, 1024, 512, elem_step=256,
                             queue_num=0)
            return gt

        seq = [(h, k) for h in range(2) for k in range(NK)]
        gts = {0: issue_gather(*seq[0], 0)}
        for i, (half, k) in enumerate(seq):
            if k == 0:
                patT = patp.tile([128, NK * 2048], F16, tag="patT")
                patTs.append(patT)
            if i + 1 < len(seq):
                gts[i + 1] = issue_gather(*seq[i + 1], i + 1)
            gt = gts.pop(i)
            if True:
                for tq in range(4):       # groups of 4 tiles per PSUM bank
                    ptr = ps_tr.tile([128, 512], F32, tag="ptr")
                    for ti in range(4):
                        tl = 4 * tq + ti
                        t = half * 16 + tl
                        wsl = w4v[:, t, k, :]
                        deng = s if tl == 6 else v
                        D = ppp.tile([128, 4, 128], F16, tag="pp")
                        for ci in range(4):
                            if deng is s:
                                s.activation(D[:, ci, :], ident_sb[:],
                                             AFT.Copy, bias=0.0,
                                             scale=wsl[:, ci: ci + 1])
                            else:
                                deng.tensor_scalar(D[:, ci, :], ident_sb[:],
                                                   wsl[:, ci: ci + 1], None,
                                                   AOT.mult)
                        for ci in range(4):
                            nc.tensor.matmul(
                                ptr[:, 128 * ti: 128 * (ti + 1)],
                                gt[:, tl, 128 * ci: 128 * (ci + 1)],
                                D[:, ci, :],
                                start=(ci == 0), stop=(ci == 3))
                    s.activation(
                        patT[:, 2048 * k + 512 * tq:
                             2048 * k + 512 * tq + 512],
                        ptr[:], AFT.Copy, bias=0.0)
                if half == 1 and k >= 1:
                    mm_group(0, patTs[0], k - 1)
                if half == 1 and k == 7:
                    # pre-accumulate taps 0..7 for half-1 groups 0..3 while
                    # tap 8 is still in flight
                    partials = []
                    for gi in range(4):
                        n, M = gi // 2, gi % 2
                        py_ = ps_y.tile([128, 512], F32, tag="ps_y")
                        partials.append(py_)
                        for kk in range(8):
                            nc.tensor.matmul(
                                py_[:],
                                dw_sb[:, COUT * kk + 128 * M:
                                      COUT * kk + 128 * (M + 1)],
                                patT[:, 2048 * kk + 512 * n:
                                     2048 * kk + 512 * (n + 1)],
                                start=(kk == 0), stop=False)
        for gi in range(4):
            n, M = gi // 2, gi % 2
            py_ = partials[gi]
            nc.tensor.matmul(
                py_[:],
                dw_sb[:, COUT * 8 + 128 * M: COUT * 8 + 128 * (M + 1)],
                patTs[1][:, 2048 * 8 + 512 * n: 2048 * 8 + 512 * (n + 1)],
                start=False, stop=True)
            ncol = 4 + n
            dst = ysb[:, HW * M + 512 * ncol: HW * M + 512 * (ncol + 1)]
            s.activation(dst, py_[:], AFT.Copy, bias=0.0,
                         accum_out=stats[:, 8 * M + ncol: 8 * M + ncol + 1])
            v.scalar_tensor_tensor(
                sq_scr[:], dst, 1.0, dst, AOT.mult, AOT.mult,
                accum_out=stats[:, 16 + 8 * M + ncol: 16 + 8 * M + ncol + 1])
        for gi in range(4, 8):
            mm_group(1, patTs[1], gi)

            if dbg_t and half == 1:
                nc.sync.dma_start(dbg_t["d_patT"].ap(), patT[:])

        if dbg_t:
            nc.sync.dma_start(dbg_t["d_offT"].ap(), offT[:])
            nc.sync.dma_start(dbg_t["d_w4"].ap(), w4[:])
            nc.sync.dma_start(dbg_t["d_ic"].ap(), icomp[:])
            nc.sync.dma_start(dbg_t["d_ig"].ap(), igath[0:128, :])
            nc.sync.dma_start(dbg_t["d_ysb"].ap(), ysb[:])

        # ---- S10: stats -> allreduce -> scale/shift ----
        st4 = pool.tile([128, 4], F32)
        stv = stats[:].rearrange("p (a n) -> p a n", n=8)
        for a in range(4):
            v.tensor_reduce(st4[:, a:a + 1], stv[:, a, :],
                            mybir.AxisListType.X, AOT.add)

        if n_cores > 1:
            nc.sync.dma_start(cc_in.ap(), st4[:])
            g.collective_compute(
                "AllReduce", AOT.add, replica_groups=[list(range(n_cores))],
                ins=[cc_in.ap()], outs=[cc_out.ap()])
            nc.sync.dma_start(st4[:], cc_out.ap())

        NTOT = float(n_cores * HW)
        mean2 = pool.tile([128, 2], F32)
        var2 = pool.tile([128, 2], F32)
        rstd2 = pool.tile([128, 2], F32)
        nsc = pool.tile([128, 2], F32)
        v.tensor_scalar(mean2[:], st4[:, 0:2], 1.0 / NTOT, None, AOT.mult)
        v.tensor_scalar(var2[:], st4[:, 2:4], 1.0 / NTOT, None, AOT.mult)
        v.tensor_tensor(rstd2[:], mean2[:], mean2[:], AOT.mult)
        v.tensor_tensor(var2[:], var2[:], rstd2[:], AOT.subtract)
        v.tensor_scalar(var2[:], var2[:], EPS, None, AOT.add)
        # rstd = 1/sqrt(var) via Newton (avoids act-table reload for Sqrt):
        # x <- x*(1.5 - 0.5*v*x^2), seed 0.25 converges for var in (0, 48)
        v.memset(rstd2[:], 0.25)
        for _ in range(6):
            v.tensor_tensor(nsc[:], rstd2[:], rstd2[:], AOT.mult)
            v.tensor_tensor(nsc[:], nsc[:], var2[:], AOT.mult)
            v.tensor_scalar(nsc[:], nsc[:], -0.5, 1.5, AOT.mult, AOT.add)
            v.tensor_tensor(rstd2[:], rstd2[:], nsc[:], AOT.mult)
        scl = pool.tile([128, 2], F32)
        sft = pool.tile([128, 2], F32)
        v.tensor_tensor(scl[:], gamma_sb[:], rstd2[:], AOT.mult)
        v.tensor_tensor(sft[:], mean2[:], scl[:], AOT.mult)
        v.tensor_tensor(sft[:], beta_sb[:], sft[:], AOT.subtract)

        # ---- S11: normalize + SiLU + output ----
        CH = HW // 2
        for M in range(2):
            yfin = pool.tile([128, HW], F16, tag="yfin")
            for c_ in range(2):
                sl = slice(CH * c_, CH * (c_ + 1))
                ysl = ysb[:, HW * M + CH * c_: HW * M + CH * (c_ + 1)]
                if use_silu:
                    s.activation(yfin[:, sl], ysl, AFT.Silu,
                                 bias=sft[:, M:M + 1], scale=scl[:, M:M + 1])
                else:  # CoreSim has no Silu; z * sigmoid(z) fallback
                    zsc = gtp.tile([128, HW], F16, tag="gt")
                    v.tensor_scalar(zsc[:, sl], ysl, scl[:, M:M + 1],
                                    sft[:, M:M + 1], AOT.mult, AOT.add)
                    s.activation(yfin[:, sl], zsc[:, sl], AFT.Sigmoid,
                                 bias=0.0)
                    v.tensor_tensor(yfin[:, sl], zsc[:, sl], yfin[:, sl],
                                    AOT.mult)
                (nc.sync if (2 * M + c_) % 2 == 0 else nc.scalar).dma_start(
                    bass.AP(tensor=yout, offset=M * 128 * HW + CH * c_,
                            ap=[[HW, 128], [1, CH]]),
                    yfin[:, sl])


# =========================================================
# host side
# =========================================================
_NC_CACHE = {}


def _get_nc(n_cores):
    if n_cores not in _NC_CACHE:
        _NC_CACHE[n_cores] = build_nc(n_cores)
    return _NC_CACHE[n_cores]


def make_in_maps(x, offset_w, offset_b, dconv_w, dconv_b, bn_gamma, bn_beta,
                 n_cores=8):
    x = np.asarray(x, np.float32)
    ow = np.asarray(offset_w, np.float32)
    dw = np.asarray(dconv_w, np.float32)
    ow_t = np.ascontiguousarray(
        ow.reshape(18, 128, 9).transpose(2, 1, 0)).astype(np.float16)
    dw_t = np.ascontiguousarray(
        dw.reshape(COUT, 128, 9).transpose(2, 1, 0)).astype(np.float16)
    ob = np.asarray(offset_b, np.float32).reshape(18, 1).copy()
    p = np.arange(128)
    t = np.arange(NT)
    k = np.arange(NK)
    ky, kx = k // 3, k % 3
    baseY = ((t[None, :, None] * 2 + (p[:, None, None] // 64)) - 1
             + ky[None, None, :]).reshape(128, TK).astype(np.float32)
    baseX = (((p[:, None, None] % 64)) - 1
             + kx[None, None, :] + 0 * t[None, :, None]).reshape(
                 128, TK).astype(np.float32)
    baseY = np.ascontiguousarray(baseY)
    baseX = np.ascontiguousarray(baseX)
    ident = np.eye(128, dtype=np.float16)
    gamma2 = np.ascontiguousarray(
        np.asarray(bn_gamma, np.float32).reshape(2, 128).T)
    beta2 = np.ascontiguousarray(
        np.asarray(bn_beta, np.float32).reshape(2, 128).T)
    gnp = mybir.dt.np(GDT)

    in_maps = []
    for c in range(n_cores):
        xb = x[c]
        xp = np.zeros((128, 66, 66), np.float16)
        xp[:, 1:65, 1:65] = xb.astype(np.float16)
        xT = np.ascontiguousarray(xb.reshape(128, HW).T)  # [pos, ch] f32
        xtp = np.zeros((HW, 256), np.float32)
        xtp[:, 0:128] = xT
        xtp[:-64, 128:256] = xT[64:]
        in_maps.append({
            "xpad": np.ascontiguousarray(xp.reshape(128, 66 * 66)),
            "xtp": xtp.astype(gnp),
            "ow_t": ow_t, "ob": ob, "dw_t": dw_t,
            "baseY": baseY, "baseX": baseX, "ident": ident,
            "gamma2": gamma2, "beta2": beta2,
        })
    return in_maps


def kernel(x, offset_w, offset_b, dconv_w, dconv_b, bn_gamma, bn_beta,
           trace=False):
    n_cores = 8
    nc = _get_nc(n_cores)
    in_maps = make_in_maps(x, offset_w, offset_b, dconv_w, dconv_b,
                           bn_gamma, bn_beta, n_cores)
    res = run_bass_kernel_spmd(nc, in_maps, list(range(n_cores)), trace=trace)
    out = np.stack([res.results[c]["yout"].reshape(COUT, H, W)
                    for c in range(n_cores)])
    kernel.last_result = res
    return out.astype(np.float32)
